# revision 1
# baseline (speedup 1.0000x reference)
"""LayerNorm-GRU Trainium2 kernel.

B=64, T=512, D=256, H=512. Data-parallel over batch: 8 rows per core, 8 cores.

Per-core device program:
  Phase 1: x-side projections z = x @ [W_xr|W_xu|W_xc] in row-major layout,
           LayerNorm (bn_stats + fused tensor_scalar apply), then PE-transpose
           into DRAM zx laid out [12 feat-tiles, 128, T*8] (features on
           partitions, (t, b) on free dim).
  Phase 2: serial recurrence in transposed layout. Per step:
           - PE: z = Wh^T-tiles stationary x h_prev moving -> [128, (12, 8)] PSUM,
             plus two folded sum-columns giving per-batch feature-sums (LN means).
           - ACT square + PE ones-matmul -> per-batch sum(z^2) (LN variance),
           - tiny DVE stats, ACT sqrt, DVE reciprocal,
           - PE 1-row matmul broadcasts per-batch stats to 128 partitions,
           - DVE/ACT: normalize, gates (sigmoid/tanh), h_new.
           Output h_t accumulates in SBUF, PE-transposed back to row-major and
           DMA'd out every 16 steps.
"""

import os
import sys

for _p in ("/opt/trn_rl_repo", "/root/.axon_site/_ro/trn_rl_repo"):
    if os.path.isdir(_p) and _p not in sys.path:
        sys.path.insert(0, _p)

import numpy as np
from contextlib import ExitStack

import concourse.bass as bass
import concourse.mybir as mybir
import concourse.tile as tile
from concourse import bacc
from concourse.bass import ds
from concourse.bass_utils import run_bass_kernel_spmd

F32 = mybir.dt.float32
AX = mybir.AxisListType
OP = mybir.AluOpType
AF = mybir.ActivationFunctionType

B, T, D, H = 64, 512, 256, 512
NCORES = 8
BL = B // NCORES          # 8 batch rows per core
H3 = 3 * H                # 1536
NT = H3 // 128            # 12 feature tiles
NRU = (2 * H) // 128      # 8 tiles in the r|u LN group
NC_ = H // 128            # 4 tiles in the c LN group
KH = H // 128             # 4 contraction chunks for the h-matmul
ROWS = T * BL             # 4096 rows (t-major: row = t*BL + b)
EPS = 1e-5

STEPS_PER_BODY = 128      # two 64-step xfeed chunks per For_i body
BLK = 16                  # hist flush granularity
CHUNK = 64                # steps per xfeed chunk


def _build_program(general_ln: bool, debug_zx: bool = False, sim_steps=None):
    nc = bacc.Bacc("TRN2", target_bir_lowering=False, debug=False)

    # ---- DRAM parameters (per-core views, replicated weights) ----
    xT_d = nc.dram_tensor("xT", [D, ROWS], F32, kind="ExternalInput")
    wx_d = nc.dram_tensor("wx", [D, H3], F32, kind="ExternalInput")
    wh_d = nc.dram_tensor("wh", [H, H3], F32, kind="ExternalInput")
    whsum_d = nc.dram_tensor("whsum", [H, 2], F32, kind="ExternalInput")
    h0t_d = nc.dram_tensor("h0t", [128, KH * BL], F32, kind="ExternalInput")
    ident_d = nc.dram_tensor("ident", [128, 128], F32, kind="ExternalInput")
    ones1_d = nc.dram_tensor("ones1", [1, 128], F32, kind="ExternalInput")
    invn_d = nc.dram_tensor("invn", [1, 16], F32, kind="ExternalInput")
    if general_ln:
        # x-side LN scale/bias expanded over partitions [128, 1536]
        gx_d = nc.dram_tensor("gx", [128, H3], F32, kind="ExternalInput")
        bx_d = nc.dram_tensor("bx", [128, H3], F32, kind="ExternalInput")
        # h-side LN scale/bias in transposed layout [128, 12]
        gh_d = nc.dram_tensor("gh", [128, NT], F32, kind="ExternalInput")
        bh_d = nc.dram_tensor("bh", [128, NT], F32, kind="ExternalInput")
    out_d = nc.dram_tensor("out", [BL, T, H], F32, kind="ExternalOutput")
    # zx: x-side LN'd projections, transposed: [feat_tile, 128, ROWS] (+pad chunk)
    zx_d = nc.dram_tensor("zx", [NT, 128, ROWS + CHUNK * BL], F32,
                          kind="ExternalOutput" if debug_zx else "Internal")

    with tile.TileContext(nc) as tc, ExitStack() as ctx:
        const_pool = ctx.enter_context(tc.tile_pool(name="consts", bufs=1))
        whs = const_pool.tile([128, KH, H3], F32)        # Wh stationaries
        whsums = const_pool.tile([128, KH, 2], F32)      # folded sum columns
        identity = const_pool.tile([128, 128], F32)
        onescol = const_pool.tile([128, 1], F32)
        ones1 = const_pool.tile([1, 128], F32)
        invn = const_pool.tile([1, 16], F32)
        epsc = const_pool.tile([128, 1], F32)
        h0t = const_pool.tile([128, KH, BL], F32)
        if general_ln:
            gx = const_pool.tile([128, H3], F32)
            bx = const_pool.tile([128, H3], F32)
            gh = const_pool.tile([128, NT], F32)
            bh = const_pool.tile([128, NT], F32)

        nc.sync.dma_start(whs[:], wh_d[:].rearrange("(k p) n -> p k n", p=128))
        nc.sync.dma_start(whsums[:], whsum_d[:].rearrange("(k p) n -> p k n", p=128))
        nc.sync.dma_start(identity[:], ident_d[:])
        nc.sync.dma_start(ones1[:], ones1_d[:])
        nc.sync.dma_start(invn[:], invn_d[:])
        nc.sync.dma_start(h0t[:], h0t_d[:].rearrange("p (k b) -> p k b", k=KH))
        nc.vector.memset(onescol[:], 1.0)
        nc.vector.memset(epsc[:], EPS)
        if general_ln:
            nc.sync.dma_start(gx[:], gx_d[:])
            nc.sync.dma_start(bx[:], bx_d[:])
            nc.sync.dma_start(gh[:], gh_d[:])
            nc.sync.dma_start(bh[:], bh_d[:])

        # ================= Phase 1: x-side projections =================
        with tc.tile_pool(name="p1sbuf", bufs=1) as p1pool, \
             tc.tile_pool(name="p1work", bufs=3) as p1work, \
             tc.tile_pool(name="p1z", bufs=2, space="PSUM") as p1z, \
             tc.tile_pool(name="p1t", bufs=2, space="PSUM") as p1t:
            xts = p1pool.tile([128, 2, ROWS], F32)
            wxs = p1pool.tile([128, 2, H3], F32)
            nc.sync.dma_start(xts[:], xT_d[:].rearrange("(k p) n -> p k n", p=128))
            nc.sync.dma_start(wxs[:], wx_d[:].rearrange("(k p) n -> p k n", p=128))

            for r in range(ROWS // 128):
                zp = p1z.tile([128, H3], F32, tag="zp")
                for k in range(2):
                    for nb in range(3):
                        nc.tensor.matmul(
                            zp[:, nb * 512:(nb + 1) * 512],
                            xts[:, k, r * 128:(r + 1) * 128],
                            wxs[:, k, nb * 512:(nb + 1) * 512],
                            start=(k == 0), stop=(k == 1),
                        )
                # LayerNorm over features, per row (partition): groups ru / c
                sixes = p1work.tile([128, 3, 6], F32, tag="sixes")
                aggr = p1work.tile([128, 2, 2], F32, tag="aggr")
                nc.vector.bn_stats(sixes[:, 0, :], zp[:, 0:512])
                nc.vector.bn_stats(sixes[:, 1, :], zp[:, 512:1024])
                nc.vector.bn_stats(sixes[:, 2, :], zp[:, 1024:1536])
                nc.vector.bn_aggr(aggr[:, 0, :], sixes[:, 0:2, :])
                nc.vector.bn_aggr(aggr[:, 1, :], sixes[:, 2, :])
                sd = p1work.tile([128, 2], F32, tag="sd")
                inv = p1work.tile([128, 2], F32, tag="inv")
                nc.scalar.activation(sd[:], aggr[:, :, 1], AF.Sqrt, bias=epsc[:])
                nc.vector.reciprocal(inv[:], sd[:])
                zln = p1work.tile([128, H3], F32, tag="zln")
                nc.vector.tensor_scalar(
                    zln[:, 0:1024], zp[:, 0:1024],
                    aggr[:, 0, 0:1], inv[:, 0:1], OP.subtract, OP.mult)
                nc.vector.tensor_scalar(
                    zln[:, 1024:1536], zp[:, 1024:1536],
                    aggr[:, 1, 0:1], inv[:, 1:2], OP.subtract, OP.mult)
                if general_ln:
                    nc.vector.tensor_mul(zln[:], zln[:], gx[:])
                    nc.vector.tensor_add(zln[:], zln[:], bx[:])
                # transpose 12 tiles to [feat, row] and stage for DMA
                ztp = p1work.tile([128, NT, 128], F32, tag="ztp")
                for m in range(NT):
                    tp = p1t.tile([128, 128], F32, tag="tp")
                    nc.tensor.transpose(tp[:], zln[:, m * 128:(m + 1) * 128],
                                        identity[:])
                    if m % 2 == 0:
                        nc.scalar.copy(ztp[:, m, :], tp[:])
                    else:
                        nc.vector.tensor_copy(ztp[:, m, :], tp[:])
                nc.sync.dma_start(
                    zx_d[:, :, r * 128:(r + 1) * 128].transpose([1, 0, 2]),
                    ztp[:])

        # ================= Phase 2: recurrence =================
        xfA = const_pool.tile([128, NT, CHUNK * BL], F32)
        xfB = const_pool.tile([128, NT, CHUNK * BL], F32)
        histP = const_pool.tile([128, KH, BLK, BL], F32)
        histQ = const_pool.tile([128, KH, BLK, BL], F32)
        obuf = const_pool.tile([128, KH, 128], F32)

        # h0 -> histQ slot 15 (step 0 reads it)
        nc.vector.tensor_copy(histQ[:, :, BLK - 1, :], h0t[:])
        # prologue: chunk 0 -> xfA
        nc.sync.dma_start(
            xfA[:], zx_d[:, :, 0:CHUNK * BL].transpose([1, 0, 2]))

        zpool = ctx.enter_context(tc.tile_pool(name="zp2", bufs=2, space="PSUM"))
        spool = ctx.enter_context(tc.tile_pool(name="sp2", bufs=1, space="PSUM"))
        bpool = ctx.enter_context(tc.tile_pool(name="bp2", bufs=2, space="PSUM"))
        tpool = ctx.enter_context(tc.tile_pool(name="tp2", bufs=1, space="PSUM"))
        wpool = ctx.enter_context(tc.tile_pool(name="w2", bufs=3))

        def emit_step(h_prev, h_out, xf, cstep):
            """One GRU step. h_prev/h_out: [128, KH, BL] APs (feat-transposed).
            xf: xfeed buffer; cstep: step index within the 64-step chunk."""
            # ru group first: its stats chain overlaps PE computing the c group
            zpru = zpool.tile([128, (NRU + 1) * BL], F32, tag="zru")
            zpc = zpool.tile([128, (NC_ + 1) * BL], F32, tag="zc")
            for m in range(NRU):
                for k in range(KH):
                    nc.tensor.matmul(
                        zpru[:, m * BL:(m + 1) * BL],
                        whs[:, k, m * 128:(m + 1) * 128],
                        h_prev[:, k, :], start=(k == 0), stop=(k == KH - 1))
            for k in range(KH):
                nc.tensor.matmul(
                    zpru[0:1, NRU * BL:(NRU + 1) * BL], whsums[:, k, 0:1],
                    h_prev[:, k, :], start=(k == 0), stop=(k == KH - 1))
            for m in range(NC_):
                for k in range(KH):
                    nc.tensor.matmul(
                        zpc[:, m * BL:(m + 1) * BL],
                        whs[:, k, (NRU + m) * 128:(NRU + m + 1) * 128],
                        h_prev[:, k, :], start=(k == 0), stop=(k == KH - 1))
            for k in range(KH):
                nc.tensor.matmul(
                    zpc[0:1, NC_ * BL:(NC_ + 1) * BL], whsums[:, k, 1:2],
                    h_prev[:, k, :], start=(k == 0), stop=(k == KH - 1))

            # ---- per-group stats chains (ru starts while c matmuls run)
            sq = wpool.tile([128, NT * BL], F32, tag="sq")
            nc.scalar.activation(sq[:, 0:NRU * BL], zpru[:, 0:NRU * BL], AF.Square)
            nc.scalar.activation(sq[:, NRU * BL:NT * BL], zpc[:, 0:NC_ * BL],
                                 AF.Square)
            # off-chain PSUM->SBUF copies of z for the later apply stage
            zS = wpool.tile([128, NT * BL], F32, tag="zS")
            nc.scalar.copy(zS[:, 0:NRU * BL], zpru[:, 0:NRU * BL])
            nc.scalar.copy(zS[:, NRU * BL:NT * BL], zpc[:, 0:NC_ * BL])
            # group-summed sum(z^2) directly via accumulating ones-matmuls
            s2 = spool.tile([1, 2, BL], F32, tag="s2")
            for m in range(NRU):
                nc.tensor.matmul(s2[:, 0, :], onescol[:],
                                 sq[:, m * BL:(m + 1) * BL],
                                 start=(m == 0), stop=(m == NRU - 1))
            for m in range(NC_):
                nc.tensor.matmul(s2[:, 1, :], onescol[:],
                                 sq[:, (NRU + m) * BL:(NRU + m + 1) * BL],
                                 start=(m == 0), stop=(m == NC_ - 1))

            stats = wpool.tile([1, 8, BL], F32, tag="stats")
            mv = wpool.tile([1, 2, BL], F32, tag="mv")
            msq = wpool.tile([1, 2, BL], F32, tag="msq")
            sd = wpool.tile([1, 2, BL], F32, tag="sdv")
            zsums = (zpru[0:1, NRU * BL:(NRU + 1) * BL],
                     zpc[0:1, NC_ * BL:(NC_ + 1) * BL])
            for g in (0, 1):
                n_feat = float(2 * H if g == 0 else H)
                nc.vector.tensor_scalar(
                    stats[:, 2 + g, :], zsums[g], 1.0 / n_feat, None, OP.mult)
                nc.vector.tensor_mul(msq[:, g, :], stats[:, 2 + g, :],
                                     stats[:, 2 + g, :])
                # var = s2/N - mean^2 fused
                nc.vector.scalar_tensor_tensor(
                    mv[:, g, :], s2[:, g, :], 1.0 / n_feat, msq[:, g, :],
                    OP.mult, OP.subtract)
                nc.scalar.activation(sd[:, g, :], mv[:, g, :], AF.Sqrt,
                                     bias=epsc[0:1, :])
                nc.vector.reciprocal(stats[:, 4 + 2 * g, :], sd[:, g, :])
                nc.vector.tensor_mul(stats[:, 5 + 2 * g, :], stats[:, 2 + g, :],
                                     stats[:, 4 + 2 * g, :])
            # broadcast per group so the ru apply does not wait on c stats
            # bc rows: 0=is_ru, 1=mis_ru, 2=is_c, 3=mis_c
            bc = bpool.tile([128, 4, BL], F32, tag="bc")
            nc.tensor.matmul(
                bc[:, 0:2, :].rearrange("p a b -> p (a b)"), ones1[:],
                stats[:, 4:6, :].rearrange("p a b -> p (a b)"),
                start=True, stop=True)
            nc.tensor.matmul(
                bc[:, 2:4, :].rearrange("p a b -> p (a b)"), ones1[:],
                stats[:, 6:8, :].rearrange("p a b -> p (a b)"),
                start=True, stop=True)
            # ---- normalize + gates (zS is SBUF, bc stays in PSUM)
            zSv = zS[:].rearrange("p (t b) -> p t b", b=BL)
            tru = wpool.tile([128, NRU, BL], F32, tag="tru")
            nc.vector.tensor_tensor(
                tru[:], zSv[:, 0:NRU, :],
                bc[:, 0:1, :].to_broadcast([128, NRU, BL]), OP.mult)
            oru = wpool.tile([128, NRU, BL], F32, tag="oru")
            nc.vector.tensor_tensor(
                oru[:], tru[:],
                bc[:, 1:2, :].to_broadcast([128, NRU, BL]), OP.subtract)
            tc_ = wpool.tile([128, NC_, BL], F32, tag="tc_")
            nc.vector.tensor_tensor(
                tc_[:], zSv[:, NRU:NT, :],
                bc[:, 2:3, :].to_broadcast([128, NC_, BL]), OP.mult)
            oc = wpool.tile([128, NC_, BL], F32, tag="oc")
            nc.vector.tensor_tensor(
                oc[:], tc_[:],
                bc[:, 3:4, :].to_broadcast([128, NC_, BL]), OP.subtract)
            if general_ln:
                nc.vector.tensor_mul(
                    oru[:], oru[:],
                    gh[:, 0:NRU].unsqueeze(2).to_broadcast([128, NRU, BL]))
                nc.vector.tensor_add(
                    oru[:], oru[:],
                    bh[:, 0:NRU].unsqueeze(2).to_broadcast([128, NRU, BL]))
                nc.vector.tensor_mul(
                    oc[:], oc[:],
                    gh[:, NRU:NT].unsqueeze(2).to_broadcast([128, NC_, BL]))
                nc.vector.tensor_add(
                    oc[:], oc[:],
                    bh[:, NRU:NT].unsqueeze(2).to_broadcast([128, NC_, BL]))

            xs = xf[:, :, cstep * BL:(cstep + 1) * BL]
            pre = wpool.tile([128, NRU, BL], F32, tag="pre")
            nc.vector.tensor_add(pre[:], oru[:], xs[:, 0:NRU, :])
            sig = wpool.tile([128, NRU, BL], F32, tag="sig")
            nc.scalar.activation(
                sig[:].rearrange("p a b -> p (a b)"),
                pre[:].rearrange("p a b -> p (a b)"), AF.Sigmoid)
            up = wpool.tile([128, NC_, BL], F32, tag="up")   # 1 - u
            nc.scalar.activation(
                up[:].rearrange("p a b -> p (a b)"),
                pre[:, NC_:NRU, :].rearrange("p a b -> p (a b)"),
                AF.Sigmoid, scale=-1.0)
            rh = wpool.tile([128, NC_, BL], F32, tag="rh")
            nc.vector.tensor_mul(rh[:], sig[:, 0:NC_, :], oc[:])
            prec = wpool.tile([128, NC_, BL], F32, tag="prec")
            nc.vector.tensor_add(prec[:], rh[:], xs[:, NRU:NT, :])
            cc = wpool.tile([128, NC_, BL], F32, tag="cc")
            nc.scalar.activation(
                cc[:].rearrange("p a b -> p (a b)"),
                prec[:].rearrange("p a b -> p (a b)"), AF.Tanh)
            a1 = wpool.tile([128, KH, BL], F32, tag="a1")
            nc.vector.tensor_mul(a1[:], up[:], h_prev)
            a2 = wpool.tile([128, KH, BL], F32, tag="a2")
            nc.vector.tensor_mul(a2[:], sig[:, NC_:NRU, :], cc[:])
            nc.vector.tensor_add(h_out, a1[:], a2[:])

        def flush_block(hist, tb_expr):
            """PE-transpose hist [128, KH, BLK, BL] back to row-major, DMA out."""
            for k in range(KH):
                tp = tpool.tile([128, 128], F32, tag="ftp")
                nc.tensor.transpose(
                    tp[:], hist[:, k, :, :], identity[:])
                if k % 2 == 0:
                    nc.scalar.copy(obuf[:, k, :], tp[:])
                else:
                    nc.vector.tensor_copy(obuf[:, k, :], tp[:])
            nc.sync.dma_start(
                out_d[:, ds(tb_expr, BLK), :].transpose([1, 0, 2]),
                obuf[:].rearrange("p k n -> p (k n)"))

        def _emit_body(ib):

            # prefetch chunk for second half of this body, and first half of next
            nc.sync.dma_start(
                xfB[:],
                zx_d[:, :, ds((ib + CHUNK) * BL, CHUNK * BL)].transpose([1, 0, 2]))
            for half in range(2):
                xf = (xfA, xfB)[half]
                for blk in range(4):
                    gblk = half * 4 + blk
                    hist = (histP, histQ)[gblk % 2]
                    prev_hist = (histP, histQ)[(gblk + 1) % 2]
                    for s in range(BLK):
                        cstep = blk * BLK + s
                        h_prev = (hist[:, :, s - 1, :] if s > 0
                                  else prev_hist[:, :, BLK - 1, :])
                        emit_step(h_prev, hist[:, :, s, :], xf, cstep)
                    flush_block(hist, ib + gblk * BLK)
            nc.sync.dma_start(
                xfA[:],
                zx_d[:, :, ds((ib + 2 * CHUNK) * BL, CHUNK * BL)].transpose([1, 0, 2]))

        if sim_steps is not None:
            for ib2 in range(0, sim_steps, STEPS_PER_BODY):
                _emit_body(ib2)
        else:
            with tc.For_i(0, T, STEPS_PER_BODY,
                          hint_engines=(mybir.EngineType.PE,
                                        mybir.EngineType.DVE,
                                        mybir.EngineType.Activation)) as ib:
                _emit_body(ib)

    nc.compile()
    return nc


_CACHE = {}
LAST_RESULT = None


def _get_program(general_ln: bool):
    if general_ln not in _CACHE:
        _CACHE[general_ln] = _build_program(general_ln)
    return _CACHE[general_ln]


def build_in_maps(inputs):
    return _prep(**inputs)[0]


def _prep(x, W_xr, W_xu, W_xc, W_hr, W_hu, W_hc, h0,
          ln_xru_scale, ln_xru_bias, ln_hru_scale, ln_hru_bias,
          ln_xc_scale, ln_xc_bias, ln_hc_scale, ln_hc_bias):
    x = np.ascontiguousarray(np.asarray(x, np.float32))
    wx = np.concatenate([W_xr, W_xu, W_xc], axis=1).astype(np.float32)
    wh = np.concatenate([W_hr, W_hu, W_hc], axis=1).astype(np.float32)
    whsum = np.stack([wh[:, :2 * H].sum(1), wh[:, 2 * H:].sum(1)], axis=1)
    whsum = np.ascontiguousarray(whsum, np.float32)

    gx_full = np.concatenate([ln_xru_scale, ln_xc_scale]).astype(np.float32)
    bx_full = np.concatenate([ln_xru_bias, ln_xc_bias]).astype(np.float32)
    gh_full = np.concatenate([ln_hru_scale, ln_hc_scale]).astype(np.float32)
    bh_full = np.concatenate([ln_hru_bias, ln_hc_bias]).astype(np.float32)
    general_ln = not (np.all(gx_full == 1) and np.all(bx_full == 0)
                      and np.all(gh_full == 1) and np.all(bh_full == 0))

    h0 = np.asarray(h0, np.float32)
    h0t = np.repeat(h0.reshape(KH, 128).T[:, :, None], BL, axis=2)
    h0t = np.ascontiguousarray(h0t.reshape(128, KH * BL), np.float32)

    ident = np.eye(128, dtype=np.float32)
    ones1 = np.ones((1, 128), np.float32)
    invn = np.concatenate([np.full(8, 1.0 / (2 * H), np.float32),
                           np.full(8, 1.0 / H, np.float32)]).reshape(1, 16)

    shared = {
        "wx": np.ascontiguousarray(wx), "wh": np.ascontiguousarray(wh),
        "whsum": whsum, "h0t": h0t, "ident": ident, "ones1": ones1,
        "invn": invn,
    }
    if general_ln:
        shared["gx"] = np.broadcast_to(gx_full, (128, H3)).copy()
        shared["bx"] = np.broadcast_to(bx_full, (128, H3)).copy()
        # transposed layout: [p, tile] where feature = tile*128 + p
        shared["gh"] = np.ascontiguousarray(gh_full.reshape(NT, 128).T)
        shared["bh"] = np.ascontiguousarray(bh_full.reshape(NT, 128).T)

    in_maps = []
    for c in range(NCORES):
        xl = x[c * BL:(c + 1) * BL]                      # [BL, T, D]
        xT = np.ascontiguousarray(
            xl.transpose(2, 1, 0).reshape(D, ROWS), np.float32)
        in_maps.append({"xT": xT, **shared})

    return in_maps, general_ln


def kernel(**inputs):
    in_maps, general_ln = _prep(**inputs)
    nc = _get_program(general_ln)
    res = run_bass_kernel_spmd(nc, in_maps, list(range(NCORES)))
    global LAST_RESULT
    LAST_RESULT = res
    outs = [res.results[c]["out"] for c in range(NCORES)]
    return np.concatenate(outs, axis=0).astype(np.float32)


if __name__ == "__main__":
    rng = np.random.default_rng(0)
    ins = {
        "x": rng.standard_normal((B, T, D), dtype=np.float32),
        "W_xr": rng.standard_normal((D, H), dtype=np.float32) / np.sqrt(D),
        "W_xu": rng.standard_normal((D, H), dtype=np.float32) / np.sqrt(D),
        "W_xc": rng.standard_normal((D, H), dtype=np.float32) / np.sqrt(D),
        "W_hr": rng.standard_normal((H, H), dtype=np.float32) / np.sqrt(H),
        "W_hu": rng.standard_normal((H, H), dtype=np.float32) / np.sqrt(H),
        "W_hc": rng.standard_normal((H, H), dtype=np.float32) / np.sqrt(H),
        "h0": np.zeros(H, np.float32),
        "ln_xru_scale": np.ones(2 * H, np.float32),
        "ln_xru_bias": np.zeros(2 * H, np.float32),
        "ln_hru_scale": np.ones(2 * H, np.float32),
        "ln_hru_bias": np.zeros(2 * H, np.float32),
        "ln_xc_scale": np.ones(H, np.float32),
        "ln_xc_bias": np.zeros(H, np.float32),
        "ln_hc_scale": np.ones(H, np.float32),
        "ln_hc_bias": np.zeros(H, np.float32),
    }
    out = kernel(**ins)
    print(out.shape, out.dtype, np.abs(out).mean())



# revision 2
# speedup vs baseline: 1.7673x; 1.7673x over previous
"""LayerNorm-GRU Trainium2 kernel, v2.

B=64, T=512, D=256, H=512. Data-parallel over batch: 8 rows/core x 8 cores.

Phase 1: x-side projections in fp32r (full fp32 data, 1 cyc/row on PE),
         LayerNorm, PE-transpose to DRAM zx [12, 128, T*8] feature-major.
Phase 2: recurrence, feature-major, single stream of 8 batch rows.
         Per step:
         - PE: 48 bf16 matmuls (stationary weight tiles [128,128] bf16 ->
           fast-weight-load), ru tiles first then c tiles, z in PSUM.
         - DVE: z PSUM->SBUF copies (per group, so the ru chain starts
           before the c matmuls finish).
         - POOL: square, tile-tree partial sums, partition_all_reduce,
           scalar stats chain with quake-rsqrt (no ACT table: only
           square/copy/sigmoid/tanh used -> one table set, zero reloads).
         - DVE: normalize + gate arithmetic; ACT: sigmoid / tanh.
"""

import os
import sys

for _p in ("/opt/trn_rl_repo", "/root/.axon_site/_ro/trn_rl_repo"):
    if os.path.isdir(_p) and _p not in sys.path:
        sys.path.insert(0, _p)

import numpy as np
import ml_dtypes
from contextlib import ExitStack

import concourse.bass as bass
import concourse.mybir as mybir
import concourse.tile as tile
from concourse import bacc
from concourse.bass import ds
from concourse.bass_utils import run_bass_kernel_spmd

F32 = mybir.dt.float32
F32R = mybir.dt.float32r
BF16 = mybir.dt.bfloat16
I32 = mybir.dt.int32
AX = mybir.AxisListType
OP = mybir.AluOpType
AF = mybir.ActivationFunctionType
RED = bass.bass_isa.ReduceOp

B, T, D, H = 64, 512, 256, 512
NCORES = 8
BL = B // NCORES          # 8 batch rows per core
H3 = 3 * H                # 1536
NT = H3 // 128            # 12 feature tiles
NRU = (2 * H) // 128      # 8 tiles in the r|u LN group
NC_ = H // 128            # 4 tiles in the c LN group
KH = H // 128             # 4 contraction chunks for the h-matmul
ROWS = T * BL             # 4096 rows (t-major: row = t*BL + b)
EPS = 1e-5

STEPS_PER_BODY = 128
BLK = 16                  # hist flush granularity
CHUNK = 64                # steps per xfeed chunk

MAGIC = 0x5F3759DF        # quake rsqrt seed constant
NEWTON_ITERS = 2

# engine for the scalar stats chain and for the gate arithmetic
CHAIN_ENGINE = "pool"     # 'pool' | 'vector'
APPLY_ENGINE = "pool"     # 'pool' | 'vector'


def _build_program(general_ln: bool, sim_steps=None,
                   chain_engine=CHAIN_ENGINE, apply_engine=APPLY_ENGINE,
                   newton_iters=NEWTON_ITERS):
    nc = bacc.Bacc("TRN2", target_bir_lowering=False, debug=False)

    xT_d = nc.dram_tensor("xT", [D, ROWS], F32R, kind="ExternalInput")
    wx_d = nc.dram_tensor("wx", [D, H3], F32R, kind="ExternalInput")
    whb_d = nc.dram_tensor("whb", [H, H3], BF16, kind="ExternalInput")
    h0t_d = nc.dram_tensor("h0t", [128, KH * BL], F32, kind="ExternalInput")
    ident_d = nc.dram_tensor("ident", [128, 128], F32, kind="ExternalInput")
    if general_ln:
        gx_d = nc.dram_tensor("gx", [128, H3], F32, kind="ExternalInput")
        bx_d = nc.dram_tensor("bx", [128, H3], F32, kind="ExternalInput")
        gh_d = nc.dram_tensor("gh", [128, NT], F32, kind="ExternalInput")
        bh_d = nc.dram_tensor("bh", [128, NT], F32, kind="ExternalInput")
    out_d = nc.dram_tensor("out", [BL, T, H], F32, kind="ExternalOutput")
    zx_d = nc.dram_tensor("zx", [NT, 128, ROWS + CHUNK * BL], F32,
                          kind="Internal")

    with tile.TileContext(nc) as tc, ExitStack() as ctx:
        const_pool = ctx.enter_context(tc.tile_pool(name="consts", bufs=1))
        whs = const_pool.tile([128, KH, H3], BF16)
        identity = const_pool.tile([128, 128], F32)
        epsc = const_pool.tile([128, 1], F32)
        h0t = const_pool.tile([128, KH, BL], F32)
        if general_ln:
            gx = const_pool.tile([128, H3], F32)
            bx = const_pool.tile([128, H3], F32)
            gh = const_pool.tile([128, NT], F32)
            bh = const_pool.tile([128, NT], F32)

        nc.sync.dma_start(whs[:], whb_d[:].rearrange("(k p) n -> p k n", p=128))
        nc.sync.dma_start(identity[:], ident_d[:])
        nc.sync.dma_start(h0t[:], h0t_d[:].rearrange("p (k b) -> p k b", k=KH))
        nc.vector.memset(epsc[:], EPS)
        if general_ln:
            nc.sync.dma_start(gx[:], gx_d[:])
            nc.sync.dma_start(bx[:], bx_d[:])
            nc.sync.dma_start(gh[:], gh_d[:])
            nc.sync.dma_start(bh[:], bh_d[:])

        # ================= Phase 1: x-side projections =================
        with tc.tile_pool(name="p1sbuf", bufs=1) as p1pool, \
             tc.tile_pool(name="p1work", bufs=3) as p1work, \
             tc.tile_pool(name="p1z", bufs=2, space="PSUM") as p1z, \
             tc.tile_pool(name="p1t", bufs=2, space="PSUM") as p1t:
            xts = p1pool.tile([128, 2, ROWS], F32R)
            wxs = p1pool.tile([128, 2, H3], F32R)
            nc.sync.dma_start(xts[:], xT_d[:].rearrange("(k p) n -> p k n", p=128))
            nc.sync.dma_start(wxs[:], wx_d[:].rearrange("(k p) n -> p k n", p=128))

            for r in range(ROWS // 128):
                zp = p1z.tile([128, H3], F32, tag="zp")
                for k in range(2):
                    for nb in range(3):
                        nc.tensor.matmul(
                            zp[:, nb * 512:(nb + 1) * 512],
                            xts[:, k, r * 128:(r + 1) * 128],
                            wxs[:, k, nb * 512:(nb + 1) * 512],
                            start=(k == 0), stop=(k == 1),
                        )
                sixes = p1work.tile([128, 3, 6], F32, tag="sixes")
                aggr = p1work.tile([128, 2, 2], F32, tag="aggr")
                nc.vector.bn_stats(sixes[:, 0, :], zp[:, 0:512])
                nc.vector.bn_stats(sixes[:, 1, :], zp[:, 512:1024])
                nc.vector.bn_stats(sixes[:, 2, :], zp[:, 1024:1536])
                nc.vector.bn_aggr(aggr[:, 0, :], sixes[:, 0:2, :])
                nc.vector.bn_aggr(aggr[:, 1, :], sixes[:, 2, :])
                sd = p1work.tile([128, 2], F32, tag="sd")
                inv = p1work.tile([128, 2], F32, tag="inv")
                nc.scalar.activation(sd[:], aggr[:, :, 1], AF.Sqrt, bias=epsc[:])
                nc.vector.reciprocal(inv[:], sd[:])
                zln = p1work.tile([128, H3], F32, tag="zln")
                nc.vector.tensor_scalar(
                    zln[:, 0:1024], zp[:, 0:1024],
                    aggr[:, 0, 0:1], inv[:, 0:1], OP.subtract, OP.mult)
                nc.vector.tensor_scalar(
                    zln[:, 1024:1536], zp[:, 1024:1536],
                    aggr[:, 1, 0:1], inv[:, 1:2], OP.subtract, OP.mult)
                if general_ln:
                    nc.vector.tensor_mul(zln[:], zln[:], gx[:])
                    nc.vector.tensor_add(zln[:], zln[:], bx[:])
                if r % 2 == 0:
                    ztp = p1work.tile([128, NT, 2, 128], F32, tag="ztp")
                for m in range(NT):
                    tp = p1t.tile([128, 128], F32, tag="tp")
                    nc.tensor.transpose(tp[:], zln[:, m * 128:(m + 1) * 128],
                                        identity[:])
                    if m % 2 == 0:
                        nc.scalar.copy(ztp[:, m, r % 2, :], tp[:])
                    else:
                        nc.vector.tensor_copy(ztp[:, m, r % 2, :], tp[:])
                if r % 2 == 1:
                    nc.sync.dma_start(
                        zx_d[:, :, (r - 1) * 128:(r + 1) * 128]
                        .transpose([1, 0, 2]),
                        ztp[:].rearrange("p t two n -> p t (two n)"))

        # ================= Phase 2: recurrence =================
        xfA = const_pool.tile([128, NT, CHUNK * BL], F32)
        xfB = const_pool.tile([128, NT, CHUNK * BL], F32)
        histP = const_pool.tile([128, KH, BLK, BL], F32)
        histQ = const_pool.tile([128, KH, BLK, BL], F32)
        obuf = const_pool.tile([128, KH, 128], F32)

        nc.vector.tensor_copy(histQ[:, :, BLK - 1, :], h0t[:])
        nc.sync.dma_start(
            xfA[:], zx_d[:, :, 0:CHUNK * BL].transpose([1, 0, 2]))

        zpool = ctx.enter_context(tc.tile_pool(name="zp2", bufs=2, space="PSUM"))
        tpool = ctx.enter_context(tc.tile_pool(name="tp2", bufs=2, space="PSUM"))
        wpool = ctx.enter_context(tc.tile_pool(name="w2", bufs=3))
        hpool = ctx.enter_context(tc.tile_pool(name="hb2", bufs=3))

        ceng = {"pool": nc.gpsimd, "vector": nc.vector}[chain_engine]
        aeng = {"pool": nc.gpsimd, "vector": nc.vector}[apply_engine]

        def group_chain(g, zq, n_feat, ntiles):
            """Stats for one LN group: tree partial sums, all-reduce,
            mean/var/quake-rsqrt. zq: SBUF [128, 2, ntiles, BL] (z | z^2).
            Returns (y, mis) each [128, BL] replicated on all partitions."""
            # tile-tree partial sums on the chain engine
            cur = zq
            nt = ntiles
            lvl = 0
            while nt > 1:
                nxt = wpool.tile([128, 2, nt // 2, BL], F32,
                                 tag=f"tr{g}_{lvl}")
                ceng.tensor_tensor(nxt[:], cur[:, :, 0:nt // 2, :],
                                   cur[:, :, nt // 2:nt, :], OP.add)
                cur = nxt
                nt //= 2
                lvl += 1
            # all-reduce across partitions: [128, 2, 1, BL] -> full sums
            allr = wpool.tile([128, 2, BL], F32, tag=f"allr{g}")
            nc.gpsimd.partition_all_reduce(
                allr[:].rearrange("p c b -> p (c b)"),
                cur[:].rearrange("p c o b -> p (c o b)"),
                channels=128, reduce_op=RED.add)
            mm = wpool.tile([128, BL], F32, tag=f"mm{g}")
            ceng.tensor_scalar(mm[:], allr[:, 0, :], 1.0 / n_feat, None,
                               OP.mult)
            msq = wpool.tile([128, BL], F32, tag=f"msq{g}")
            ceng.tensor_tensor(msq[:], mm[:], mm[:], OP.mult)
            ve = wpool.tile([128, BL], F32, tag=f"ve{g}")
            ceng.tensor_scalar(ve[:], allr[:, 1, :], 1.0 / n_feat, EPS,
                               OP.mult, OP.add)
            v = wpool.tile([128, BL], F32, tag=f"v{g}")
            ceng.tensor_tensor(v[:], ve[:], msq[:], OP.subtract)
            # seed: bitwise ops are illegal on Pool -> one fused DVE op
            # computes ~(i >> 1); Pool then adds MAGIC+1 (int add is legal),
            # giving MAGIC - (i >> 1).
            nt_ = wpool.tile([128, BL], I32, tag=f"nt{g}")
            nc.vector.tensor_scalar(nt_[:], v[:].bitcast(I32), 1, -1,
                                    OP.logical_shift_right, OP.bitwise_xor)
            y = wpool.tile([128, BL], F32, tag=f"y{g}")
            ceng.tensor_scalar(y[:].bitcast(I32), nt_[:], MAGIC + 1, None,
                               OP.add)
            for it in range(newton_iters):
                a = wpool.tile([128, BL], F32, tag=f"qa{g}_{it}")
                ceng.tensor_tensor(a[:], y[:], y[:], OP.mult)
                w_ = wpool.tile([128, BL], F32, tag=f"qw{g}_{it}")
                ceng.tensor_tensor(w_[:], v[:], a[:], OP.mult)
                f_ = wpool.tile([128, BL], F32, tag=f"qf{g}_{it}")
                ceng.tensor_scalar(f_[:], w_[:], -0.5, 1.5, OP.mult, OP.add)
                y2 = wpool.tile([128, BL], F32, tag=f"qy{g}_{it}")
                ceng.tensor_tensor(y2[:], y[:], f_[:], OP.mult)
                y = y2
            mis = wpool.tile([128, BL], F32, tag=f"mis{g}")
            ceng.tensor_tensor(mis[:], mm[:], y[:], OP.mult)
            return y, mis

        def emit_step(h_prev, h_out, hb_prev, xf, cstep):
            """One GRU step. h_prev/h_out: [128, KH, BL] APs (feature-major).
            hb_prev: [128, KH, BL] bf16 tile; returns the next hb tile."""
            zru = zpool.tile([128, NRU, BL], F32, tag="zru")
            zc = zpool.tile([128, NC_, BL], F32, tag="zc")
            for m in range(NRU):
                for k in range(KH):
                    nc.tensor.matmul(
                        zru[:, m, :], whs[:, k, m * 128:(m + 1) * 128],
                        hb_prev[:, k, :], start=(k == 0), stop=(k == KH - 1))
            for m in range(NC_):
                for k in range(KH):
                    nc.tensor.matmul(
                        zc[:, m, :], whs[:, k, (NRU + m) * 128:(NRU + m + 1) * 128],
                        hb_prev[:, k, :], start=(k == 0), stop=(k == KH - 1))

            # bridge PSUM -> SBUF (DVE), per group; square on chain engine
            zqru = wpool.tile([128, 2, NRU, BL], F32, tag="zqru")
            nc.vector.tensor_copy(
                zqru[:, 0, :, :].rearrange("p t b -> p (t b)"),
                zru[:].rearrange("p t b -> p (t b)"))
            ceng.tensor_tensor(zqru[:, 1, :, :], zqru[:, 0, :, :],
                               zqru[:, 0, :, :], OP.mult)
            zqc = wpool.tile([128, 2, NC_, BL], F32, tag="zqc")
            nc.vector.tensor_copy(
                zqc[:, 0, :, :].rearrange("p t b -> p (t b)"),
                zc[:].rearrange("p t b -> p (t b)"))
            ceng.tensor_tensor(zqc[:, 1, :, :], zqc[:, 0, :, :],
                               zqc[:, 0, :, :], OP.mult)

            y_ru, mis_ru = group_chain("r", zqru, 2.0 * H, NRU)
            y_c, mis_c = group_chain("c", zqc, float(H), NC_)

            xs = xf[:, :, cstep * BL:(cstep + 1) * BL]
            # ru apply: pre = z*is + (x - mis)
            xm = wpool.tile([128, NRU, BL], F32, tag="xm")
            aeng.tensor_tensor(
                xm[:], xs[:, 0:NRU, :],
                mis_ru[:].unsqueeze(1).to_broadcast([128, NRU, BL]),
                OP.subtract)
            tru = wpool.tile([128, NRU, BL], F32, tag="tru")
            aeng.tensor_tensor(
                tru[:], zqru[:, 0, :, :],
                y_ru[:].unsqueeze(1).to_broadcast([128, NRU, BL]), OP.mult)
            if general_ln:
                nc.vector.tensor_mul(
                    tru[:], tru[:],
                    gh[:, 0:NRU].unsqueeze(2).to_broadcast([128, NRU, BL]))
                gmis = wpool.tile([128, NRU, BL], F32, tag="gmis")
                nc.vector.tensor_tensor(
                    gmis[:],
                    mis_ru[:].unsqueeze(1).to_broadcast([128, NRU, BL]),
                    gh[:, 0:NRU].unsqueeze(2).to_broadcast([128, NRU, BL]),
                    OP.mult)
                nc.vector.tensor_tensor(
                    xm[:], xs[:, 0:NRU, :], gmis[:], OP.subtract)
                nc.vector.tensor_add(
                    xm[:], xm[:],
                    bh[:, 0:NRU].unsqueeze(2).to_broadcast([128, NRU, BL]))
            pre = wpool.tile([128, NRU, BL], F32, tag="pre")
            aeng.tensor_tensor(pre[:], tru[:], xm[:], OP.add)
            sig = wpool.tile([128, NRU, BL], F32, tag="sig")
            nc.scalar.activation(
                sig[:].rearrange("p a b -> p (a b)"),
                pre[:].rearrange("p a b -> p (a b)"), AF.Sigmoid)
            # c apply
            tc_ = wpool.tile([128, NC_, BL], F32, tag="tc_")
            aeng.tensor_tensor(
                tc_[:], zqc[:, 0, :, :],
                y_c[:].unsqueeze(1).to_broadcast([128, NC_, BL]), OP.mult)
            oc = wpool.tile([128, NC_, BL], F32, tag="oc")
            aeng.tensor_tensor(
                oc[:], tc_[:],
                mis_c[:].unsqueeze(1).to_broadcast([128, NC_, BL]),
                OP.subtract)
            if general_ln:
                nc.vector.tensor_mul(
                    oc[:], oc[:],
                    gh[:, NRU:NT].unsqueeze(2).to_broadcast([128, NC_, BL]))
                nc.vector.tensor_add(
                    oc[:], oc[:],
                    bh[:, NRU:NT].unsqueeze(2).to_broadcast([128, NC_, BL]))
            rh = wpool.tile([128, NC_, BL], F32, tag="rh")
            aeng.tensor_tensor(rh[:], sig[:, 0:NC_, :], oc[:], OP.mult)
            prec = wpool.tile([128, NC_, BL], F32, tag="prec")
            aeng.tensor_tensor(prec[:], rh[:], xs[:, NRU:NT, :], OP.add)
            cc = wpool.tile([128, NC_, BL], F32, tag="cc")
            nc.scalar.activation(
                cc[:].rearrange("p a b -> p (a b)"),
                prec[:].rearrange("p a b -> p (a b)"), AF.Tanh)
            dd = wpool.tile([128, KH, BL], F32, tag="dd")
            aeng.tensor_tensor(dd[:], cc[:], h_prev, OP.subtract)
            ud = wpool.tile([128, KH, BL], F32, tag="ud")
            aeng.tensor_tensor(ud[:], sig[:, NC_:NRU, :], dd[:], OP.mult)
            aeng.tensor_tensor(h_out, h_prev, ud[:], OP.add)
            hb = hpool.tile([128, KH, BL], BF16, tag="hb")
            aeng.tensor_copy(hb[:], h_out)
            return hb

        def flush_block(hist, tb_expr):
            for k in range(KH):
                tp = tpool.tile([128, 128], F32, tag="ftp")
                nc.tensor.transpose(tp[:], hist[:, k, :, :], identity[:])
                if k % 2 == 0:
                    nc.scalar.copy(obuf[:, k, :], tp[:])
                else:
                    nc.vector.tensor_copy(obuf[:, k, :], tp[:])
            nc.sync.dma_start(
                out_d[:, ds(tb_expr, BLK), :].transpose([1, 0, 2]),
                obuf[:].rearrange("p k n -> p (k n)"))

        def _emit_body(ib):
            hb = hpool.tile([128, KH, BL], BF16, tag="hb")
            nc.vector.tensor_copy(hb[:], histQ[:, :, BLK - 1, :])
            nc.sync.dma_start(
                xfB[:],
                zx_d[:, :, ds((ib + CHUNK) * BL, CHUNK * BL)].transpose([1, 0, 2]))
            for half in range(2):
                xf = (xfA, xfB)[half]
                for blk in range(4):
                    gblk = half * 4 + blk
                    hist = (histP, histQ)[gblk % 2]
                    prev_hist = (histP, histQ)[(gblk + 1) % 2]
                    for s in range(BLK):
                        cstep = blk * BLK + s
                        h_prev = (hist[:, :, s - 1, :] if s > 0
                                  else prev_hist[:, :, BLK - 1, :])
                        hb = emit_step(h_prev, hist[:, :, s, :], hb, xf, cstep)
                    flush_block(hist, ib + gblk * BLK)
            nc.sync.dma_start(
                xfA[:],
                zx_d[:, :, ds((ib + 2 * CHUNK) * BL, CHUNK * BL)].transpose([1, 0, 2]))

        if sim_steps is not None:
            for ib2 in range(0, sim_steps, STEPS_PER_BODY):
                _emit_body(ib2)
        else:
            with tc.For_i(0, T, STEPS_PER_BODY,
                          hint_engines=(mybir.EngineType.PE,
                                        mybir.EngineType.DVE,
                                        mybir.EngineType.Activation,
                                        mybir.EngineType.Pool)) as ib:
                _emit_body(ib)

    nc.compile()
    return nc


_CACHE = {}
LAST_RESULT = None


def _get_program(general_ln: bool):
    if general_ln not in _CACHE:
        _CACHE[general_ln] = _build_program(general_ln)
    return _CACHE[general_ln]


def build_in_maps(inputs):
    return _prep(**inputs)[0]


def _prep(x, W_xr, W_xu, W_xc, W_hr, W_hu, W_hc, h0,
          ln_xru_scale, ln_xru_bias, ln_hru_scale, ln_hru_bias,
          ln_xc_scale, ln_xc_bias, ln_hc_scale, ln_hc_bias):
    x = np.ascontiguousarray(np.asarray(x, np.float32))
    wx = np.concatenate([W_xr, W_xu, W_xc], axis=1).astype(np.float32)
    wh = np.concatenate([W_hr, W_hu, W_hc], axis=1).astype(np.float32)
    whb = np.ascontiguousarray(wh.astype(ml_dtypes.bfloat16))

    gx_full = np.concatenate([ln_xru_scale, ln_xc_scale]).astype(np.float32)
    bx_full = np.concatenate([ln_xru_bias, ln_xc_bias]).astype(np.float32)
    gh_full = np.concatenate([ln_hru_scale, ln_hc_scale]).astype(np.float32)
    bh_full = np.concatenate([ln_hru_bias, ln_hc_bias]).astype(np.float32)
    general_ln = not (np.all(gx_full == 1) and np.all(bx_full == 0)
                      and np.all(gh_full == 1) and np.all(bh_full == 0))

    h0 = np.asarray(h0, np.float32)
    h0t = np.repeat(h0.reshape(KH, 128).T[:, :, None], BL, axis=2)
    h0t = np.ascontiguousarray(h0t.reshape(128, KH * BL), np.float32)

    ident = np.eye(128, dtype=np.float32)

    shared = {
        "wx": np.ascontiguousarray(wx), "whb": whb,
        "h0t": h0t, "ident": ident,
    }
    if general_ln:
        shared["gx"] = np.broadcast_to(gx_full, (128, H3)).copy()
        shared["bx"] = np.broadcast_to(bx_full, (128, H3)).copy()
        shared["gh"] = np.ascontiguousarray(gh_full.reshape(NT, 128).T)
        shared["bh"] = np.ascontiguousarray(bh_full.reshape(NT, 128).T)

    in_maps = []
    for c in range(NCORES):
        xl = x[c * BL:(c + 1) * BL]                      # [BL, T, D]
        xT = np.ascontiguousarray(
            xl.transpose(2, 1, 0).reshape(D, ROWS), np.float32)
        in_maps.append({"xT": xT, **shared})

    return in_maps, general_ln


def kernel(**inputs):
    in_maps, general_ln = _prep(**inputs)
    nc = _get_program(general_ln)
    res = run_bass_kernel_spmd(nc, in_maps, list(range(NCORES)))
    global LAST_RESULT
    LAST_RESULT = res
    outs = [res.results[c]["out"] for c in range(NCORES)]
    return np.concatenate(outs, axis=0).astype(np.float32)


if __name__ == "__main__":
    rng = np.random.default_rng(0)
    ins = {
        "x": rng.standard_normal((B, T, D), dtype=np.float32),
        "W_xr": rng.standard_normal((D, H), dtype=np.float32) / np.sqrt(D),
        "W_xu": rng.standard_normal((D, H), dtype=np.float32) / np.sqrt(D),
        "W_xc": rng.standard_normal((D, H), dtype=np.float32) / np.sqrt(D),
        "W_hr": rng.standard_normal((H, H), dtype=np.float32) / np.sqrt(H),
        "W_hu": rng.standard_normal((H, H), dtype=np.float32) / np.sqrt(H),
        "W_hc": rng.standard_normal((H, H), dtype=np.float32) / np.sqrt(H),
        "h0": np.zeros(H, np.float32),
        "ln_xru_scale": np.ones(2 * H, np.float32),
        "ln_xru_bias": np.zeros(2 * H, np.float32),
        "ln_hru_scale": np.ones(2 * H, np.float32),
        "ln_hru_bias": np.zeros(2 * H, np.float32),
        "ln_xc_scale": np.ones(H, np.float32),
        "ln_xc_bias": np.zeros(H, np.float32),
        "ln_hc_scale": np.ones(H, np.float32),
        "ln_hc_bias": np.zeros(H, np.float32),
    }
    out = kernel(**ins)
    print(out.shape, out.dtype, np.abs(out).mean())


# revision 3
# speedup vs baseline: 1.7764x; 1.0051x over previous
"""LayerNorm-GRU Trainium2 kernel, v2.

B=64, T=512, D=256, H=512. Data-parallel over batch: 8 rows/core x 8 cores.

Phase 1: x-side projections in fp32r (full fp32 data, 1 cyc/row on PE),
         LayerNorm (bn_stats), PE-transpose to DRAM zx [12, 128, T*8]
         feature-major.
Phase 2: recurrence, feature-major, 8 batch rows per core. Per step:
         - PE: 48 bf16 matmuls (stationary weight tiles [128,128] bf16 ->
           fast-weight-load), ru tiles first then c tiles, z in PSUM.
         - bridge: z copy PSUM->SBUF on DVE, z^2 via ACT Square (parallel);
           per LN group so the ru chain starts before the c matmuls finish.
         - stats: DVE strided reduce over feature tiles, then (stats_engine
           'pe') a ones-column matmul for the cross-partition sums, a tiny
           DVE chain computing mean/var and 1/sqrt(var+eps) via the quake
           bitwise seed + one Newton step (no ACT Sqrt -> the single
           sigmoid/tanh/square/copy table set stays resident, zero table
           reloads), and a 1x128 ones matmul broadcasting the per-batch
           stats to all partitions.
         - apply/gates: DVE normalize + gate arithmetic; ACT sigmoid/tanh.
         Output h_t accumulates in SBUF, PE-transposed to row-major and
         DMA'd out every 16 steps.
"""

import os
import sys

for _p in ("/opt/trn_rl_repo", "/root/.axon_site/_ro/trn_rl_repo"):
    if os.path.isdir(_p) and _p not in sys.path:
        sys.path.insert(0, _p)

import numpy as np
import ml_dtypes
from contextlib import ExitStack

import concourse.bass as bass
import concourse.mybir as mybir
import concourse.tile as tile
from concourse import bacc
from concourse.bass import ds
from concourse.bass_utils import run_bass_kernel_spmd

F32 = mybir.dt.float32
F32R = mybir.dt.float32r
BF16 = mybir.dt.bfloat16
I32 = mybir.dt.int32
AX = mybir.AxisListType
OP = mybir.AluOpType
AF = mybir.ActivationFunctionType
RED = bass.bass_isa.ReduceOp

B, T, D, H = 64, 512, 256, 512
NCORES = 8
BL = B // NCORES          # 8 batch rows per core
H3 = 3 * H                # 1536
NT = H3 // 128            # 12 feature tiles
NRU = (2 * H) // 128      # 8 tiles in the r|u LN group
NC_ = H // 128            # 4 tiles in the c LN group
KH = H // 128             # 4 contraction chunks for the h-matmul
ROWS = T * BL             # 4096 rows (t-major: row = t*BL + b)
EPS = 1e-5

STEPS_PER_BODY = 128
BLK = 16                  # hist flush granularity
CHUNK = 64                # steps per xfeed chunk

MAGIC = 0x5F3759DF        # quake rsqrt seed constant
NEWTON_ITERS = 1

# engine for the scalar stats chain and for the gate arithmetic
CHAIN_ENGINE = "vector"   # 'pool' | 'vector'
APPLY_ENGINE = "vector"   # 'pool' | 'vector'
# cross-partition reduction/broadcast: gpsimd all-reduce vs PE matmuls
STATS_ENGINE = "pe"       # 'pool' | 'pe'


def _build_program(general_ln: bool, sim_steps=None,
                   chain_engine=CHAIN_ENGINE, apply_engine=APPLY_ENGINE,
                   newton_iters=NEWTON_ITERS, stats_engine=STATS_ENGINE):
    nc = bacc.Bacc("TRN2", target_bir_lowering=False, debug=False)

    xT_d = nc.dram_tensor("xT", [D, ROWS], F32R, kind="ExternalInput")
    wx_d = nc.dram_tensor("wx", [D, H3], F32R, kind="ExternalInput")
    whb_d = nc.dram_tensor("whb", [H, H3], BF16, kind="ExternalInput")
    h0t_d = nc.dram_tensor("h0t", [128, KH * BL], F32, kind="ExternalInput")
    ident_d = nc.dram_tensor("ident", [128, 128], F32, kind="ExternalInput")
    if general_ln:
        gx_d = nc.dram_tensor("gx", [128, H3], F32, kind="ExternalInput")
        bx_d = nc.dram_tensor("bx", [128, H3], F32, kind="ExternalInput")
        gh_d = nc.dram_tensor("gh", [128, NT], F32, kind="ExternalInput")
        bh_d = nc.dram_tensor("bh", [128, NT], F32, kind="ExternalInput")
    out_d = nc.dram_tensor("out", [BL, T, H], F32, kind="ExternalOutput")
    zx_d = nc.dram_tensor("zx", [NT, 128, ROWS + CHUNK * BL], F32,
                          kind="Internal")

    with tile.TileContext(nc) as tc, ExitStack() as ctx:
        const_pool = ctx.enter_context(tc.tile_pool(name="consts", bufs=1))
        whs = const_pool.tile([128, KH, H3], BF16)
        identity = const_pool.tile([128, 128], F32)
        epsc = const_pool.tile([128, 1], F32)
        h0t = const_pool.tile([128, KH, BL], F32)
        onescol = const_pool.tile([128, 1], F32)
        ones1 = const_pool.tile([1, 128], F32)
        nc.vector.memset(onescol[:], 1.0)
        nc.vector.memset(ones1[:], 1.0)
        if general_ln:
            gx = const_pool.tile([128, H3], F32)
            bx = const_pool.tile([128, H3], F32)
            gh = const_pool.tile([128, NT], F32)
            bh = const_pool.tile([128, NT], F32)

        nc.sync.dma_start(whs[:], whb_d[:].rearrange("(k p) n -> p k n", p=128))
        nc.sync.dma_start(identity[:], ident_d[:])
        nc.sync.dma_start(h0t[:], h0t_d[:].rearrange("p (k b) -> p k b", k=KH))
        nc.vector.memset(epsc[:], EPS)
        if general_ln:
            nc.sync.dma_start(gx[:], gx_d[:])
            nc.sync.dma_start(bx[:], bx_d[:])
            nc.sync.dma_start(gh[:], gh_d[:])
            nc.sync.dma_start(bh[:], bh_d[:])

        # ================= Phase 1: x-side projections =================
        with tc.tile_pool(name="p1sbuf", bufs=1) as p1pool, \
             tc.tile_pool(name="p1work", bufs=3) as p1work, \
             tc.tile_pool(name="p1z", bufs=2, space="PSUM") as p1z, \
             tc.tile_pool(name="p1t", bufs=2, space="PSUM") as p1t:
            xts = p1pool.tile([128, 2, ROWS], F32R)
            wxs = p1pool.tile([128, 2, H3], F32R)
            nc.sync.dma_start(xts[:], xT_d[:].rearrange("(k p) n -> p k n", p=128))
            nc.sync.dma_start(wxs[:], wx_d[:].rearrange("(k p) n -> p k n", p=128))

            for r in range(ROWS // 128):
                zp = p1z.tile([128, H3], F32, tag="zp")
                for k in range(2):
                    for nb in range(3):
                        nc.tensor.matmul(
                            zp[:, nb * 512:(nb + 1) * 512],
                            xts[:, k, r * 128:(r + 1) * 128],
                            wxs[:, k, nb * 512:(nb + 1) * 512],
                            start=(k == 0), stop=(k == 1),
                        )
                sixes = p1work.tile([128, 3, 6], F32, tag="sixes")
                aggr = p1work.tile([128, 2, 2], F32, tag="aggr")
                nc.vector.bn_stats(sixes[:, 0, :], zp[:, 0:512])
                nc.vector.bn_stats(sixes[:, 1, :], zp[:, 512:1024])
                nc.vector.bn_stats(sixes[:, 2, :], zp[:, 1024:1536])
                nc.vector.bn_aggr(aggr[:, 0, :], sixes[:, 0:2, :])
                nc.vector.bn_aggr(aggr[:, 1, :], sixes[:, 2, :])
                sd = p1work.tile([128, 2], F32, tag="sd")
                inv = p1work.tile([128, 2], F32, tag="inv")
                nc.scalar.activation(sd[:], aggr[:, :, 1], AF.Sqrt, bias=epsc[:])
                nc.vector.reciprocal(inv[:], sd[:])
                zln = p1work.tile([128, H3], F32, tag="zln")
                nc.vector.tensor_scalar(
                    zln[:, 0:1024], zp[:, 0:1024],
                    aggr[:, 0, 0:1], inv[:, 0:1], OP.subtract, OP.mult)
                nc.vector.tensor_scalar(
                    zln[:, 1024:1536], zp[:, 1024:1536],
                    aggr[:, 1, 0:1], inv[:, 1:2], OP.subtract, OP.mult)
                if general_ln:
                    nc.vector.tensor_mul(zln[:], zln[:], gx[:])
                    nc.vector.tensor_add(zln[:], zln[:], bx[:])
                if r % 2 == 0:
                    ztp = p1work.tile([128, NT, 2, 128], F32, tag="ztp")
                for m in range(NT):
                    tp = p1t.tile([128, 128], F32, tag="tp")
                    nc.tensor.transpose(tp[:], zln[:, m * 128:(m + 1) * 128],
                                        identity[:])
                    # DVE is the phase-1 bottleneck (bn_stats + LN apply);
                    # route most PSUM->SBUF staging copies to ACT instead.
                    if m % 4 == 3:
                        nc.vector.tensor_copy(ztp[:, m, r % 2, :], tp[:])
                    else:
                        nc.scalar.copy(ztp[:, m, r % 2, :], tp[:])
                if r % 2 == 1:
                    nc.sync.dma_start(
                        zx_d[:, :, (r - 1) * 128:(r + 1) * 128]
                        .transpose([1, 0, 2]),
                        ztp[:].rearrange("p t two n -> p t (two n)"))

        # ================= Phase 2: recurrence =================
        xfA = const_pool.tile([128, NT, CHUNK * BL], F32)
        xfB = const_pool.tile([128, NT, CHUNK * BL], F32)
        histP = const_pool.tile([128, KH, BLK, BL], F32)
        histQ = const_pool.tile([128, KH, BLK, BL], F32)
        obuf = const_pool.tile([128, KH, 128], F32)

        nc.vector.tensor_copy(histQ[:, :, BLK - 1, :], h0t[:])
        nc.sync.dma_start(
            xfA[:], zx_d[:, :, 0:CHUNK * BL].transpose([1, 0, 2]))

        zpool = ctx.enter_context(tc.tile_pool(name="zp2", bufs=2, space="PSUM"))
        spool = ctx.enter_context(tc.tile_pool(name="sp2", bufs=2, space="PSUM"))
        tpool = ctx.enter_context(tc.tile_pool(name="tp2", bufs=2, space="PSUM"))
        wpool = ctx.enter_context(tc.tile_pool(name="w2", bufs=3))
        hpool = ctx.enter_context(tc.tile_pool(name="hb2", bufs=3))

        ceng = {"pool": nc.gpsimd, "vector": nc.vector}[chain_engine]
        aeng = {"pool": nc.gpsimd, "vector": nc.vector}[apply_engine]

        def chain_ops(P, src_sums, n_feat, g, sb=None, goff=0):
            """Mean/var/quake-rsqrt on [P, BL] tiles from src_sums
            ([P, 2, BL]: z-sums | sq-sums). Returns (y_ap, mis_ap) as
            [P, BL] APs (for 'pe', written into SBUF stats tile)."""
            mm = wpool.tile([P, BL], F32, tag=f"mm{g}")
            ceng.tensor_scalar(mm[:], src_sums[:, 0, :], 1.0 / n_feat, None,
                               OP.mult)
            msq = wpool.tile([P, BL], F32, tag=f"msq{g}")
            ceng.tensor_tensor(msq[:], mm[:], mm[:], OP.mult)
            ve = wpool.tile([P, BL], F32, tag=f"ve{g}")
            ceng.tensor_scalar(ve[:], src_sums[:, 1, :], 1.0 / n_feat, EPS,
                               OP.mult, OP.add)
            v = wpool.tile([P, BL], F32, tag=f"v{g}")
            ceng.tensor_tensor(v[:], ve[:], msq[:], OP.subtract)
            # quake seed: one fused DVE op computes ~(i >> 1) (bitwise ops
            # are illegal on Pool); then an int add gives MAGIC - (i >> 1).
            nt_ = wpool.tile([P, BL], I32, tag=f"nt{g}")
            nc.vector.tensor_scalar(nt_[:], v[:].bitcast(I32), 1, -1,
                                    OP.logical_shift_right, OP.bitwise_xor)
            y_t = wpool.tile([P, BL], F32, tag=f"y{g}")
            y = y_t[:]
            ceng.tensor_scalar(y.bitcast(I32), nt_[:], MAGIC + 1, None,
                               OP.add)
            for it in range(newton_iters):
                a = wpool.tile([P, BL], F32, tag=f"qa{g}_{it}")
                ceng.tensor_tensor(a[:], y, y, OP.mult)
                w_ = wpool.tile([P, BL], F32, tag=f"qw{g}_{it}")
                ceng.tensor_tensor(w_[:], v[:], a[:], OP.mult)
                f_ = wpool.tile([P, BL], F32, tag=f"qf{g}_{it}")
                ceng.tensor_scalar(f_[:], w_[:], -0.5, 1.5, OP.mult, OP.add)
                last = it == newton_iters - 1
                if last and sb is not None:
                    y2 = sb[0:1, 0:BL]
                else:
                    y2_t = wpool.tile([P, BL], F32, tag=f"qy{g}_{it}")
                    y2 = y2_t[:]
                ceng.tensor_tensor(y2, y, f_[:], OP.mult)
                y = y2
            if sb is not None:
                mis = sb[0:1, BL:2 * BL]
            else:
                mis_t = wpool.tile([P, BL], F32, tag=f"mis{g}")
                mis = mis_t[:]
            ceng.tensor_tensor(mis, mm[:], y, OP.mult)
            return y, mis

        def group_chain(g, zq, n_feat, ntiles, sbp, goff):
            """Stats for one LN group. zq: SBUF [128, 2, ntiles, BL]
            (z | z^2). Returns (y_bc, mis_bc) as [128, BL] APs replicated
            on all partitions (SBUF for 'pool', PSUM for 'pe')."""
            ps = wpool.tile([128, 2, BL], F32, tag=f"ps{g}")
            nc.vector.tensor_reduce(
                ps[:], zq[:].rearrange("p c t b -> p c b t"), AX.X, OP.add)
            if stats_engine == "pool":
                allr = wpool.tile([128, 2, BL], F32, tag=f"allr{g}")
                nc.gpsimd.partition_all_reduce(
                    allr[:].rearrange("p c b -> p (c b)"),
                    ps[:].rearrange("p c b -> p (c b)"),
                    channels=128, reduce_op=RED.add)
                y, mis = chain_ops(128, allr, n_feat, g)
                return y, mis
            # 'pe': partition sums via ones-matmul, broadcast via 1xN matmul
            nc.tensor.matmul(
                sbp[0:1, goff:goff + 2 * BL], onescol[:, 0:1],
                ps[:].rearrange("p c b -> p (c b)"), start=True, stop=True)
            st = wpool.tile([1, 2 * BL], F32, tag=f"st{g}")
            y, mis = chain_ops(
                1, sbp[0:1, goff:goff + 2 * BL].rearrange(
                    "p (c b) -> p c b", c=2), n_feat, g, sb=st)
            nc.tensor.matmul(
                sbp[:, goff + 2 * BL:goff + 4 * BL], ones1[0:1, :], st[0:1, :],
                start=True, stop=True)
            return (sbp[:, goff + 2 * BL:goff + 3 * BL],
                    sbp[:, goff + 3 * BL:goff + 4 * BL])

        def emit_step(h_prev, h_out, hb_prev, xf, cstep):
            """One GRU step. h_prev/h_out: [128, KH, BL] APs (feature-major).
            hb_prev: [128, KH, BL] bf16 tile; returns the next hb tile."""
            zru = zpool.tile([128, NRU, BL], F32, tag="zru")
            zc = zpool.tile([128, NC_, BL], F32, tag="zc")
            for m in range(NRU):
                for k in range(KH):
                    nc.tensor.matmul(
                        zru[:, m, :], whs[:, k, m * 128:(m + 1) * 128],
                        hb_prev[:, k, :], start=(k == 0), stop=(k == KH - 1))
            for m in range(NC_):
                for k in range(KH):
                    nc.tensor.matmul(
                        zc[:, m, :], whs[:, k, (NRU + m) * 128:(NRU + m + 1) * 128],
                        hb_prev[:, k, :], start=(k == 0), stop=(k == KH - 1))

            # bridge PSUM -> SBUF: z copy on DVE, square on ACT (parallel)
            zqru = wpool.tile([128, 2, NRU, BL], F32, tag="zqru")
            nc.vector.tensor_copy(
                zqru[:, 0, :, :].rearrange("p t b -> p (t b)"),
                zru[:].rearrange("p t b -> p (t b)"))
            nc.scalar.activation(
                zqru[:, 1, :, :].rearrange("p t b -> p (t b)"),
                zru[:].rearrange("p t b -> p (t b)"), AF.Square)
            zqc = wpool.tile([128, 2, NC_, BL], F32, tag="zqc")
            nc.vector.tensor_copy(
                zqc[:, 0, :, :].rearrange("p t b -> p (t b)"),
                zc[:].rearrange("p t b -> p (t b)"))
            nc.scalar.activation(
                zqc[:, 1, :, :].rearrange("p t b -> p (t b)"),
                zc[:].rearrange("p t b -> p (t b)"), AF.Square)

            sbp = None
            if stats_engine == "pe":
                sbp = spool.tile([128, 8 * BL], F32, tag="sb")
            y_ru, mis_ru = group_chain("r", zqru, 2.0 * H, NRU, sbp, 0)
            y_c, mis_c = group_chain("c", zqc, float(H), NC_, sbp, 4 * BL)

            xs = xf[:, :, cstep * BL:(cstep + 1) * BL]
            # ru apply: pre = z*is + (x - mis)
            xm = wpool.tile([128, NRU, BL], F32, tag="xm")
            aeng.tensor_tensor(
                xm[:], xs[:, 0:NRU, :],
                mis_ru.unsqueeze(1).to_broadcast([128, NRU, BL]),
                OP.subtract)
            tru = wpool.tile([128, NRU, BL], F32, tag="tru")
            aeng.tensor_tensor(
                tru[:], zqru[:, 0, :, :],
                y_ru.unsqueeze(1).to_broadcast([128, NRU, BL]), OP.mult)
            if general_ln:
                nc.vector.tensor_mul(
                    tru[:], tru[:],
                    gh[:, 0:NRU].unsqueeze(2).to_broadcast([128, NRU, BL]))
                gmis = wpool.tile([128, NRU, BL], F32, tag="gmis")
                nc.vector.tensor_tensor(
                    gmis[:],
                    mis_ru.unsqueeze(1).to_broadcast([128, NRU, BL]),
                    gh[:, 0:NRU].unsqueeze(2).to_broadcast([128, NRU, BL]),
                    OP.mult)
                nc.vector.tensor_tensor(
                    xm[:], xs[:, 0:NRU, :], gmis[:], OP.subtract)
                nc.vector.tensor_add(
                    xm[:], xm[:],
                    bh[:, 0:NRU].unsqueeze(2).to_broadcast([128, NRU, BL]))
            pre = wpool.tile([128, NRU, BL], F32, tag="pre")
            aeng.tensor_tensor(pre[:], tru[:], xm[:], OP.add)
            sig = wpool.tile([128, NRU, BL], F32, tag="sig")
            nc.scalar.activation(
                sig[:].rearrange("p a b -> p (a b)"),
                pre[:].rearrange("p a b -> p (a b)"), AF.Sigmoid)
            # c apply
            tc_ = wpool.tile([128, NC_, BL], F32, tag="tc_")
            aeng.tensor_tensor(
                tc_[:], zqc[:, 0, :, :],
                y_c.unsqueeze(1).to_broadcast([128, NC_, BL]), OP.mult)
            oc = wpool.tile([128, NC_, BL], F32, tag="oc")
            aeng.tensor_tensor(
                oc[:], tc_[:],
                mis_c.unsqueeze(1).to_broadcast([128, NC_, BL]),
                OP.subtract)
            if general_ln:
                nc.vector.tensor_mul(
                    oc[:], oc[:],
                    gh[:, NRU:NT].unsqueeze(2).to_broadcast([128, NC_, BL]))
                nc.vector.tensor_add(
                    oc[:], oc[:],
                    bh[:, NRU:NT].unsqueeze(2).to_broadcast([128, NC_, BL]))
            rh = wpool.tile([128, NC_, BL], F32, tag="rh")
            aeng.tensor_tensor(rh[:], sig[:, 0:NC_, :], oc[:], OP.mult)
            prec = wpool.tile([128, NC_, BL], F32, tag="prec")
            aeng.tensor_tensor(prec[:], rh[:], xs[:, NRU:NT, :], OP.add)
            cc = wpool.tile([128, NC_, BL], F32, tag="cc")
            nc.scalar.activation(
                cc[:].rearrange("p a b -> p (a b)"),
                prec[:].rearrange("p a b -> p (a b)"), AF.Tanh)
            dd = wpool.tile([128, KH, BL], F32, tag="dd")
            aeng.tensor_tensor(dd[:], cc[:], h_prev, OP.subtract)
            ud = wpool.tile([128, KH, BL], F32, tag="ud")
            aeng.tensor_tensor(ud[:], sig[:, NC_:NRU, :], dd[:], OP.mult)
            aeng.tensor_tensor(h_out, h_prev, ud[:], OP.add)
            hb = hpool.tile([128, KH, BL], BF16, tag="hb")
            aeng.tensor_copy(hb[:], h_out)
            return hb

        def flush_block(hist, tb_expr):
            for k in range(KH):
                tp = tpool.tile([128, 128], F32, tag="ftp")
                nc.tensor.transpose(tp[:], hist[:, k, :, :], identity[:])
                if k % 2 == 0:
                    nc.scalar.copy(obuf[:, k, :], tp[:])
                else:
                    nc.vector.tensor_copy(obuf[:, k, :], tp[:])
            nc.sync.dma_start(
                out_d[:, ds(tb_expr, BLK), :].transpose([1, 0, 2]),
                obuf[:].rearrange("p k n -> p (k n)"))

        def _emit_body(ib):
            hb = hpool.tile([128, KH, BL], BF16, tag="hb")
            nc.vector.tensor_copy(hb[:], histQ[:, :, BLK - 1, :])
            nc.sync.dma_start(
                xfB[:],
                zx_d[:, :, ds((ib + CHUNK) * BL, CHUNK * BL)].transpose([1, 0, 2]))
            for half in range(2):
                xf = (xfA, xfB)[half]
                for blk in range(4):
                    gblk = half * 4 + blk
                    hist = (histP, histQ)[gblk % 2]
                    prev_hist = (histP, histQ)[(gblk + 1) % 2]
                    for s in range(BLK):
                        cstep = blk * BLK + s
                        h_prev = (hist[:, :, s - 1, :] if s > 0
                                  else prev_hist[:, :, BLK - 1, :])
                        hb = emit_step(h_prev, hist[:, :, s, :], hb, xf, cstep)
                    flush_block(hist, ib + gblk * BLK)
            nc.sync.dma_start(
                xfA[:],
                zx_d[:, :, ds((ib + 2 * CHUNK) * BL, CHUNK * BL)].transpose([1, 0, 2]))

        if sim_steps is not None:
            for ib2 in range(0, sim_steps, STEPS_PER_BODY):
                _emit_body(ib2)
        else:
            with tc.For_i(0, T, STEPS_PER_BODY,
                          hint_engines=(mybir.EngineType.PE,
                                        mybir.EngineType.DVE,
                                        mybir.EngineType.Activation,
                                        mybir.EngineType.Pool)) as ib:
                _emit_body(ib)

    nc.compile()
    return nc


_CACHE = {}
LAST_RESULT = None


def _get_program(general_ln: bool):
    if general_ln not in _CACHE:
        _CACHE[general_ln] = _build_program(general_ln)
    return _CACHE[general_ln]


def build_in_maps(inputs):
    return _prep(**inputs)[0]


def _prep(x, W_xr, W_xu, W_xc, W_hr, W_hu, W_hc, h0,
          ln_xru_scale, ln_xru_bias, ln_hru_scale, ln_hru_bias,
          ln_xc_scale, ln_xc_bias, ln_hc_scale, ln_hc_bias):
    x = np.ascontiguousarray(np.asarray(x, np.float32))
    wx = np.concatenate([W_xr, W_xu, W_xc], axis=1).astype(np.float32)
    wh = np.concatenate([W_hr, W_hu, W_hc], axis=1).astype(np.float32)
    whb = np.ascontiguousarray(wh.astype(ml_dtypes.bfloat16))

    gx_full = np.concatenate([ln_xru_scale, ln_xc_scale]).astype(np.float32)
    bx_full = np.concatenate([ln_xru_bias, ln_xc_bias]).astype(np.float32)
    gh_full = np.concatenate([ln_hru_scale, ln_hc_scale]).astype(np.float32)
    bh_full = np.concatenate([ln_hru_bias, ln_hc_bias]).astype(np.float32)
    general_ln = not (np.all(gx_full == 1) and np.all(bx_full == 0)
                      and np.all(gh_full == 1) and np.all(bh_full == 0))

    h0 = np.asarray(h0, np.float32)
    h0t = np.repeat(h0.reshape(KH, 128).T[:, :, None], BL, axis=2)
    h0t = np.ascontiguousarray(h0t.reshape(128, KH * BL), np.float32)

    ident = np.eye(128, dtype=np.float32)

    shared = {
        "wx": np.ascontiguousarray(wx), "whb": whb,
        "h0t": h0t, "ident": ident,
    }
    if general_ln:
        shared["gx"] = np.broadcast_to(gx_full, (128, H3)).copy()
        shared["bx"] = np.broadcast_to(bx_full, (128, H3)).copy()
        shared["gh"] = np.ascontiguousarray(gh_full.reshape(NT, 128).T)
        shared["bh"] = np.ascontiguousarray(bh_full.reshape(NT, 128).T)

    in_maps = []
    for c in range(NCORES):
        xl = x[c * BL:(c + 1) * BL]                      # [BL, T, D]
        xT = np.ascontiguousarray(
            xl.transpose(2, 1, 0).reshape(D, ROWS), np.float32)
        in_maps.append({"xT": xT, **shared})

    return in_maps, general_ln


def kernel(**inputs):
    in_maps, general_ln = _prep(**inputs)
    nc = _get_program(general_ln)
    res = run_bass_kernel_spmd(nc, in_maps, list(range(NCORES)))
    global LAST_RESULT
    LAST_RESULT = res
    outs = [res.results[c]["out"] for c in range(NCORES)]
    return np.concatenate(outs, axis=0).astype(np.float32)


if __name__ == "__main__":
    rng = np.random.default_rng(0)
    ins = {
        "x": rng.standard_normal((B, T, D), dtype=np.float32),
        "W_xr": rng.standard_normal((D, H), dtype=np.float32) / np.sqrt(D),
        "W_xu": rng.standard_normal((D, H), dtype=np.float32) / np.sqrt(D),
        "W_xc": rng.standard_normal((D, H), dtype=np.float32) / np.sqrt(D),
        "W_hr": rng.standard_normal((H, H), dtype=np.float32) / np.sqrt(H),
        "W_hu": rng.standard_normal((H, H), dtype=np.float32) / np.sqrt(H),
        "W_hc": rng.standard_normal((H, H), dtype=np.float32) / np.sqrt(H),
        "h0": np.zeros(H, np.float32),
        "ln_xru_scale": np.ones(2 * H, np.float32),
        "ln_xru_bias": np.zeros(2 * H, np.float32),
        "ln_hru_scale": np.ones(2 * H, np.float32),
        "ln_hru_bias": np.zeros(2 * H, np.float32),
        "ln_xc_scale": np.ones(H, np.float32),
        "ln_xc_bias": np.zeros(H, np.float32),
        "ln_hc_scale": np.ones(H, np.float32),
        "ln_hc_bias": np.zeros(H, np.float32),
    }
    out = kernel(**ins)
    print(out.shape, out.dtype, np.abs(out).mean())


# revision 4
# speedup vs baseline: 1.7890x; 1.0071x over previous
"""LayerNorm-GRU Trainium2 kernel, v2.

B=64, T=512, D=256, H=512. Data-parallel over batch: 8 rows/core x 8 cores.

Phase 1: x-side projections in fp32r (full fp32 data, 1 cyc/row on PE),
         LayerNorm (bn_stats), PE-transpose to DRAM zx [12, 128, T*8]
         feature-major.
Phase 2: recurrence, feature-major, 8 batch rows per core. Per step:
         - PE: 48 bf16 matmuls (stationary weight tiles [128,128] bf16 ->
           fast-weight-load), ru tiles first then c tiles, z in PSUM.
         - bridge: z copy PSUM->SBUF on DVE, z^2 via ACT Square (parallel);
           per LN group so the ru chain starts before the c matmuls finish.
         - stats: DVE strided reduce over feature tiles, then (stats_engine
           'pe') a ones-column matmul for the cross-partition sums, a tiny
           DVE chain computing mean/var and 1/sqrt(var+eps) via the quake
           bitwise seed + one Newton step (no ACT Sqrt -> the single
           sigmoid/tanh/square/copy table set stays resident, zero table
           reloads), and a 1x128 ones matmul broadcasting the per-batch
           stats to all partitions.
         - apply/gates: DVE normalize + gate arithmetic; ACT sigmoid/tanh.
         Output h_t accumulates in SBUF, PE-transposed to row-major and
         DMA'd out every 16 steps.
"""

import os
import sys

for _p in ("/opt/trn_rl_repo", "/root/.axon_site/_ro/trn_rl_repo"):
    if os.path.isdir(_p) and _p not in sys.path:
        sys.path.insert(0, _p)

import numpy as np
import ml_dtypes
from contextlib import ExitStack

import concourse.bass as bass
import concourse.mybir as mybir
import concourse.tile as tile
from concourse import bacc
from concourse.bass import ds
from concourse.bass_utils import run_bass_kernel_spmd

F32 = mybir.dt.float32
F32R = mybir.dt.float32r
BF16 = mybir.dt.bfloat16
I32 = mybir.dt.int32
AX = mybir.AxisListType
OP = mybir.AluOpType
AF = mybir.ActivationFunctionType
RED = bass.bass_isa.ReduceOp

B, T, D, H = 64, 512, 256, 512
NCORES = 8
BL = B // NCORES          # 8 batch rows per core
H3 = 3 * H                # 1536
NT = H3 // 128            # 12 feature tiles
NRU = (2 * H) // 128      # 8 tiles in the r|u LN group
NC_ = H // 128            # 4 tiles in the c LN group
KH = H // 128             # 4 contraction chunks for the h-matmul
ROWS = T * BL             # 4096 rows (t-major: row = t*BL + b)
EPS = 1e-5

STEPS_PER_BODY = 128
BLK = 16                  # hist flush granularity
CHUNK = 64                # steps per xfeed chunk

MAGIC = 0x5F3759DF        # quake rsqrt seed constant
NEWTON_ITERS = 1

# engine for the scalar stats chain and for the gate arithmetic
CHAIN_ENGINE = "vector"   # 'pool' | 'vector'
APPLY_ENGINE = "vector"   # 'pool' | 'vector'
# cross-partition reduction/broadcast: gpsimd all-reduce vs PE matmuls
STATS_ENGINE = "pe"       # 'pool' | 'pe'


def _build_program(general_ln: bool, sim_steps=None,
                   chain_engine=CHAIN_ENGINE, apply_engine=APPLY_ENGINE,
                   newton_iters=NEWTON_ITERS, stats_engine=STATS_ENGINE):
    nc = bacc.Bacc("TRN2", target_bir_lowering=False, debug=False)

    xT_d = nc.dram_tensor("xT", [D, ROWS], F32R, kind="ExternalInput")
    wx_d = nc.dram_tensor("wx", [D, H3], F32R, kind="ExternalInput")
    whb_d = nc.dram_tensor("whb", [H, H3], BF16, kind="ExternalInput")
    whsb_d = nc.dram_tensor("whsb", [H, 2], BF16, kind="ExternalInput")
    h0t_d = nc.dram_tensor("h0t", [128, KH * BL], F32, kind="ExternalInput")
    ident_d = nc.dram_tensor("ident", [128, 128], F32, kind="ExternalInput")
    if general_ln:
        gx_d = nc.dram_tensor("gx", [128, H3], F32, kind="ExternalInput")
        bx_d = nc.dram_tensor("bx", [128, H3], F32, kind="ExternalInput")
        gh_d = nc.dram_tensor("gh", [128, NT], F32, kind="ExternalInput")
        bh_d = nc.dram_tensor("bh", [128, NT], F32, kind="ExternalInput")
    out_d = nc.dram_tensor("out", [BL, T, H], F32, kind="ExternalOutput")
    zx_d = nc.dram_tensor("zx", [NT, 128, ROWS + CHUNK * BL], F32,
                          kind="Internal")

    with tile.TileContext(nc) as tc, ExitStack() as ctx:
        const_pool = ctx.enter_context(tc.tile_pool(name="consts", bufs=1))
        whs = const_pool.tile([128, KH, H3], BF16)
        identity = const_pool.tile([128, 128], F32)
        epsc = const_pool.tile([128, 1], F32)
        h0t = const_pool.tile([128, KH, BL], F32)
        onescol = const_pool.tile([128, 1], F32)
        ones1 = const_pool.tile([1, 128], F32)
        onescl = const_pool.tile([128, 2], F32)   # 1/N per LN group
        whsums = const_pool.tile([128, KH, 2], BF16)
        nc.vector.memset(onescol[:], 1.0)
        nc.vector.memset(ones1[:], 1.0)
        nc.vector.memset(onescl[:, 0:1], 1.0 / (2 * H))
        nc.vector.memset(onescl[:, 1:2], 1.0 / H)
        nc.sync.dma_start(whsums[:],
                          whsb_d[:].rearrange("(k p) n -> p k n", p=128))
        if general_ln:
            gx = const_pool.tile([128, H3], F32)
            bx = const_pool.tile([128, H3], F32)
            gh = const_pool.tile([128, NT], F32)
            bh = const_pool.tile([128, NT], F32)

        nc.sync.dma_start(whs[:], whb_d[:].rearrange("(k p) n -> p k n", p=128))
        nc.sync.dma_start(identity[:], ident_d[:])
        nc.sync.dma_start(h0t[:], h0t_d[:].rearrange("p (k b) -> p k b", k=KH))
        nc.vector.memset(epsc[:], EPS)
        if general_ln:
            nc.sync.dma_start(gx[:], gx_d[:])
            nc.sync.dma_start(bx[:], bx_d[:])
            nc.sync.dma_start(gh[:], gh_d[:])
            nc.sync.dma_start(bh[:], bh_d[:])

        # ================= Phase 1: x-side projections =================
        with tc.tile_pool(name="p1sbuf", bufs=1) as p1pool, \
             tc.tile_pool(name="p1work", bufs=3) as p1work, \
             tc.tile_pool(name="p1z", bufs=2, space="PSUM") as p1z, \
             tc.tile_pool(name="p1t", bufs=2, space="PSUM") as p1t:
            xts = p1pool.tile([128, 2, ROWS], F32R)
            wxs = p1pool.tile([128, 2, H3], F32R)
            nc.sync.dma_start(xts[:], xT_d[:].rearrange("(k p) n -> p k n", p=128))
            nc.sync.dma_start(wxs[:], wx_d[:].rearrange("(k p) n -> p k n", p=128))

            for r in range(ROWS // 128):
                zp = p1z.tile([128, H3], F32, tag="zp")
                for k in range(2):
                    for nb in range(3):
                        nc.tensor.matmul(
                            zp[:, nb * 512:(nb + 1) * 512],
                            xts[:, k, r * 128:(r + 1) * 128],
                            wxs[:, k, nb * 512:(nb + 1) * 512],
                            start=(k == 0), stop=(k == 1),
                        )
                sixes = p1work.tile([128, 3, 6], F32, tag="sixes")
                aggr = p1work.tile([128, 2, 2], F32, tag="aggr")
                nc.vector.bn_stats(sixes[:, 0, :], zp[:, 0:512])
                nc.vector.bn_stats(sixes[:, 1, :], zp[:, 512:1024])
                nc.vector.bn_stats(sixes[:, 2, :], zp[:, 1024:1536])
                nc.vector.bn_aggr(aggr[:, 0, :], sixes[:, 0:2, :])
                nc.vector.bn_aggr(aggr[:, 1, :], sixes[:, 2, :])
                sd = p1work.tile([128, 2], F32, tag="sd")
                inv = p1work.tile([128, 2], F32, tag="inv")
                nc.scalar.activation(sd[:], aggr[:, :, 1], AF.Sqrt, bias=epsc[:])
                nc.vector.reciprocal(inv[:], sd[:])
                zln = p1work.tile([128, H3], F32, tag="zln")
                nc.vector.tensor_scalar(
                    zln[:, 0:1024], zp[:, 0:1024],
                    aggr[:, 0, 0:1], inv[:, 0:1], OP.subtract, OP.mult)
                nc.vector.tensor_scalar(
                    zln[:, 1024:1536], zp[:, 1024:1536],
                    aggr[:, 1, 0:1], inv[:, 1:2], OP.subtract, OP.mult)
                if general_ln:
                    nc.vector.tensor_mul(zln[:], zln[:], gx[:])
                    nc.vector.tensor_add(zln[:], zln[:], bx[:])
                if r % 2 == 0:
                    ztp = p1work.tile([128, NT, 2, 128], F32, tag="ztp")
                for m in range(NT):
                    tp = p1t.tile([128, 128], F32, tag="tp")
                    nc.tensor.transpose(tp[:], zln[:, m * 128:(m + 1) * 128],
                                        identity[:])
                    # DVE is the phase-1 bottleneck (bn_stats + LN apply);
                    # route most PSUM->SBUF staging copies to ACT instead.
                    if m % 4 == 3:
                        nc.vector.tensor_copy(ztp[:, m, r % 2, :], tp[:])
                    else:
                        nc.scalar.copy(ztp[:, m, r % 2, :], tp[:])
                if r % 2 == 1:
                    nc.sync.dma_start(
                        zx_d[:, :, (r - 1) * 128:(r + 1) * 128]
                        .transpose([1, 0, 2]),
                        ztp[:].rearrange("p t two n -> p t (two n)"))

        # ================= Phase 2: recurrence =================
        xfA = const_pool.tile([128, NT, CHUNK * BL], F32)
        xfB = const_pool.tile([128, NT, CHUNK * BL], F32)
        histP = const_pool.tile([128, KH, BLK, BL], F32)
        histQ = const_pool.tile([128, KH, BLK, BL], F32)
        obuf = const_pool.tile([128, KH, 128], F32)

        nc.vector.tensor_copy(histQ[:, :, BLK - 1, :], h0t[:])
        nc.sync.dma_start(
            xfA[:], zx_d[:, :, 0:CHUNK * BL].transpose([1, 0, 2]))

        zpool = ctx.enter_context(tc.tile_pool(name="zp2", bufs=2, space="PSUM"))
        spool = ctx.enter_context(tc.tile_pool(name="sp2", bufs=2, space="PSUM"))
        tpool = ctx.enter_context(tc.tile_pool(name="tp2", bufs=2, space="PSUM"))
        wpool = ctx.enter_context(tc.tile_pool(name="w2", bufs=3))
        hpool = ctx.enter_context(tc.tile_pool(name="hb2", bufs=3))

        ceng = {"pool": nc.gpsimd, "vector": nc.vector}[chain_engine]
        aeng = {"pool": nc.gpsimd, "vector": nc.vector}[apply_engine]

        def chain_ops(P, src_sums, n_feat, g, sb=None, goff=0):
            """Mean/var/quake-rsqrt on [P, BL] tiles from src_sums
            ([P, 2, BL]: z-sums | sq-sums). Returns (y_ap, mis_ap) as
            [P, BL] APs (for 'pe', written into SBUF stats tile)."""
            mm = wpool.tile([P, BL], F32, tag=f"mm{g}")
            ceng.tensor_scalar(mm[:], src_sums[:, 0, :], 1.0 / n_feat, None,
                               OP.mult)
            msq = wpool.tile([P, BL], F32, tag=f"msq{g}")
            ceng.tensor_tensor(msq[:], mm[:], mm[:], OP.mult)
            ve = wpool.tile([P, BL], F32, tag=f"ve{g}")
            ceng.tensor_scalar(ve[:], src_sums[:, 1, :], 1.0 / n_feat, EPS,
                               OP.mult, OP.add)
            v = wpool.tile([P, BL], F32, tag=f"v{g}")
            ceng.tensor_tensor(v[:], ve[:], msq[:], OP.subtract)
            # quake seed: one fused DVE op computes ~(i >> 1) (bitwise ops
            # are illegal on Pool); then an int add gives MAGIC - (i >> 1).
            nt_ = wpool.tile([P, BL], I32, tag=f"nt{g}")
            nc.vector.tensor_scalar(nt_[:], v[:].bitcast(I32), 1, -1,
                                    OP.logical_shift_right, OP.bitwise_xor)
            y_t = wpool.tile([P, BL], F32, tag=f"y{g}")
            y = y_t[:]
            ceng.tensor_scalar(y.bitcast(I32), nt_[:], MAGIC + 1, None,
                               OP.add)
            for it in range(newton_iters):
                a = wpool.tile([P, BL], F32, tag=f"qa{g}_{it}")
                ceng.tensor_tensor(a[:], y, y, OP.mult)
                w_ = wpool.tile([P, BL], F32, tag=f"qw{g}_{it}")
                ceng.tensor_tensor(w_[:], v[:], a[:], OP.mult)
                f_ = wpool.tile([P, BL], F32, tag=f"qf{g}_{it}")
                ceng.tensor_scalar(f_[:], w_[:], -0.5, 1.5, OP.mult, OP.add)
                last = it == newton_iters - 1
                if last and sb is not None:
                    y2 = sb[0:1, 0:BL]
                else:
                    y2_t = wpool.tile([P, BL], F32, tag=f"qy{g}_{it}")
                    y2 = y2_t[:]
                ceng.tensor_tensor(y2, y, f_[:], OP.mult)
                y = y2
            if sb is not None:
                mis = sb[0:1, BL:2 * BL]
            else:
                mis_t = wpool.tile([P, BL], F32, tag=f"mis{g}")
                mis = mis_t[:]
            ceng.tensor_tensor(mis, mm[:], y, OP.mult)
            return y, mis

        def group_chain(g, gi, zq, n_feat, ntiles, sbp):
            """Stats for one LN group. zq: SBUF [128, 2, ntiles, BL]
            (z | z^2). Returns (y_bc, mis_bc) as [128, BL] APs replicated
            on all partitions (SBUF for 'pool', PSUM for 'pe')."""
            if stats_engine == "pool":
                ps = wpool.tile([128, 2, BL], F32, tag=f"ps{g}")
                nc.vector.tensor_reduce(
                    ps[:], zq[:].rearrange("p c t b -> p c b t"), AX.X, OP.add)
                allr = wpool.tile([128, 2, BL], F32, tag=f"allr{g}")
                nc.gpsimd.partition_all_reduce(
                    allr[:].rearrange("p c b -> p (c b)"),
                    ps[:].rearrange("p c b -> p (c b)"),
                    channels=128, reduce_op=RED.add)
                y, mis = chain_ops(128, allr, n_feat, g)
                return y, mis
            # 'pe': the group mean is already accumulating in
            # sbp[0:1, gi*BL:(gi+1)*BL] via the pre-scaled folded weight
            # columns (part of the PE matmul phase); only sum(z^2) needs the
            # reduce + ones-matmul (the ones column is pre-scaled by 1/N).
            psq = wpool.tile([128, BL], F32, tag=f"ps{g}")
            nc.vector.tensor_reduce(
                psq[:], zq[:, 1, :, :].rearrange("p t b -> p b t"),
                AX.X, OP.add)
            nc.tensor.matmul(
                sbp[0:1, (2 + gi) * BL:(3 + gi) * BL], onescl[:, gi:gi + 1],
                psq[:], start=True, stop=True)
            mcp = wpool.tile([1, BL], F32, tag=f"mcp{g}")
            nc.vector.tensor_copy(mcp[:], sbp[0:1, gi * BL:(gi + 1) * BL])
            msq = wpool.tile([1, BL], F32, tag=f"msq{g}")
            ceng.tensor_tensor(msq[:], mcp[:], mcp[:], OP.mult)
            v = wpool.tile([1, BL], F32, tag=f"v{g}")
            nc.vector.scalar_tensor_tensor(
                v[:], sbp[0:1, (2 + gi) * BL:(3 + gi) * BL], EPS, msq[:],
                OP.add, OP.subtract)
            nt_ = wpool.tile([1, BL], I32, tag=f"nt{g}")
            nc.vector.tensor_scalar(nt_[:], v[:].bitcast(I32), 1, -1,
                                    OP.logical_shift_right, OP.bitwise_xor)
            st = wpool.tile([1, 2 * BL], F32, tag=f"st{g}")
            y_t = wpool.tile([1, BL], F32, tag=f"yq{g}")
            y = y_t[:]
            ceng.tensor_scalar(y.bitcast(I32), nt_[:], MAGIC + 1, None,
                               OP.add)
            for it in range(newton_iters):
                a = wpool.tile([1, BL], F32, tag=f"qa{g}_{it}")
                ceng.tensor_tensor(a[:], y, y, OP.mult)
                w_ = wpool.tile([1, BL], F32, tag=f"qw{g}_{it}")
                ceng.tensor_tensor(w_[:], v[:], a[:], OP.mult)
                f_ = wpool.tile([1, BL], F32, tag=f"qf{g}_{it}")
                ceng.tensor_scalar(f_[:], w_[:], -0.5, 1.5, OP.mult, OP.add)
                y2 = (st[0:1, 0:BL] if it == newton_iters - 1
                      else None)
                if y2 is None:
                    y2_t = wpool.tile([1, BL], F32, tag=f"qy{g}_{it}")
                    y2 = y2_t[:]
                ceng.tensor_tensor(y2, y, f_[:], OP.mult)
                y = y2
            ceng.tensor_tensor(st[0:1, BL:2 * BL], mcp[:], y, OP.mult)
            goff = (4 + 2 * gi) * BL
            nc.tensor.matmul(
                sbp[:, goff:goff + 2 * BL], ones1[0:1, :], st[0:1, :],
                start=True, stop=True)
            return (sbp[:, goff:goff + BL],
                    sbp[:, goff + BL:goff + 2 * BL])

        def emit_step(h_prev, h_out, hb_prev, xf, cstep):
            """One GRU step. h_prev/h_out: [128, KH, BL] APs (feature-major).
            hb_prev: [128, KH, BL] bf16 tile; returns the next hb tile."""
            zru = zpool.tile([128, NRU, BL], F32, tag="zru")
            zc = zpool.tile([128, NC_, BL], F32, tag="zc")
            sbp = None
            if stats_engine == "pe":
                sbp = spool.tile([128, 8 * BL], F32, tag="sb")
            for m in range(NRU):
                for k in range(KH):
                    nc.tensor.matmul(
                        zru[:, m, :], whs[:, k, m * 128:(m + 1) * 128],
                        hb_prev[:, k, :], start=(k == 0), stop=(k == KH - 1))
            if stats_engine == "pe":
                # group means ride along as two extra matmul columns against
                # the pre-scaled folded weight sums
                for gi in range(2):
                    for k in range(KH):
                        nc.tensor.matmul(
                            sbp[0:1, gi * BL:(gi + 1) * BL],
                            whsums[:, k, gi:gi + 1], hb_prev[:, k, :],
                            start=(k == 0), stop=(k == KH - 1))
            for m in range(NC_):
                for k in range(KH):
                    nc.tensor.matmul(
                        zc[:, m, :], whs[:, k, (NRU + m) * 128:(NRU + m + 1) * 128],
                        hb_prev[:, k, :], start=(k == 0), stop=(k == KH - 1))

            # bridge PSUM -> SBUF: z copy on DVE, square on ACT (parallel)
            zqru = wpool.tile([128, 2, NRU, BL], F32, tag="zqru")
            nc.vector.tensor_copy(
                zqru[:, 0, :, :].rearrange("p t b -> p (t b)"),
                zru[:].rearrange("p t b -> p (t b)"))
            nc.scalar.activation(
                zqru[:, 1, :, :].rearrange("p t b -> p (t b)"),
                zru[:].rearrange("p t b -> p (t b)"), AF.Square)
            zqc = wpool.tile([128, 2, NC_, BL], F32, tag="zqc")
            nc.vector.tensor_copy(
                zqc[:, 0, :, :].rearrange("p t b -> p (t b)"),
                zc[:].rearrange("p t b -> p (t b)"))
            nc.scalar.activation(
                zqc[:, 1, :, :].rearrange("p t b -> p (t b)"),
                zc[:].rearrange("p t b -> p (t b)"), AF.Square)

            y_ru, mis_ru = group_chain("r", 0, zqru, 2.0 * H, NRU, sbp)

            xs = xf[:, :, cstep * BL:(cstep + 1) * BL]
            # ru apply: pre = z*is + (x - mis)  (emitted BEFORE the c-group
            # chain so the scheduler runs the c chain during sigmoid, not
            # ahead of the critical ru-apply path)
            xm = wpool.tile([128, NRU, BL], F32, tag="xm")
            aeng.tensor_tensor(
                xm[:], xs[:, 0:NRU, :],
                mis_ru.unsqueeze(1).to_broadcast([128, NRU, BL]),
                OP.subtract)
            tru = wpool.tile([128, NRU, BL], F32, tag="tru")
            aeng.tensor_tensor(
                tru[:], zqru[:, 0, :, :],
                y_ru.unsqueeze(1).to_broadcast([128, NRU, BL]), OP.mult)
            if general_ln:
                nc.vector.tensor_mul(
                    tru[:], tru[:],
                    gh[:, 0:NRU].unsqueeze(2).to_broadcast([128, NRU, BL]))
                gmis = wpool.tile([128, NRU, BL], F32, tag="gmis")
                nc.vector.tensor_tensor(
                    gmis[:],
                    mis_ru.unsqueeze(1).to_broadcast([128, NRU, BL]),
                    gh[:, 0:NRU].unsqueeze(2).to_broadcast([128, NRU, BL]),
                    OP.mult)
                nc.vector.tensor_tensor(
                    xm[:], xs[:, 0:NRU, :], gmis[:], OP.subtract)
                nc.vector.tensor_add(
                    xm[:], xm[:],
                    bh[:, 0:NRU].unsqueeze(2).to_broadcast([128, NRU, BL]))
            pre = wpool.tile([128, NRU, BL], F32, tag="pre")
            aeng.tensor_tensor(pre[:], tru[:], xm[:], OP.add)
            sig = wpool.tile([128, NRU, BL], F32, tag="sig")
            nc.scalar.activation(
                sig[:].rearrange("p a b -> p (a b)"),
                pre[:].rearrange("p a b -> p (a b)"), AF.Sigmoid)

            y_c, mis_c = group_chain("c", 1, zqc, float(H), NC_, sbp)
            # c apply
            tc_ = wpool.tile([128, NC_, BL], F32, tag="tc_")
            aeng.tensor_tensor(
                tc_[:], zqc[:, 0, :, :],
                y_c.unsqueeze(1).to_broadcast([128, NC_, BL]), OP.mult)
            oc = wpool.tile([128, NC_, BL], F32, tag="oc")
            aeng.tensor_tensor(
                oc[:], tc_[:],
                mis_c.unsqueeze(1).to_broadcast([128, NC_, BL]),
                OP.subtract)
            if general_ln:
                nc.vector.tensor_mul(
                    oc[:], oc[:],
                    gh[:, NRU:NT].unsqueeze(2).to_broadcast([128, NC_, BL]))
                nc.vector.tensor_add(
                    oc[:], oc[:],
                    bh[:, NRU:NT].unsqueeze(2).to_broadcast([128, NC_, BL]))
            rh = wpool.tile([128, NC_, BL], F32, tag="rh")
            aeng.tensor_tensor(rh[:], sig[:, 0:NC_, :], oc[:], OP.mult)
            prec = wpool.tile([128, NC_, BL], F32, tag="prec")
            aeng.tensor_tensor(prec[:], rh[:], xs[:, NRU:NT, :], OP.add)
            cc = wpool.tile([128, NC_, BL], F32, tag="cc")
            nc.scalar.activation(
                cc[:].rearrange("p a b -> p (a b)"),
                prec[:].rearrange("p a b -> p (a b)"), AF.Tanh)
            dd = wpool.tile([128, KH, BL], F32, tag="dd")
            aeng.tensor_tensor(dd[:], cc[:], h_prev, OP.subtract)
            ud = wpool.tile([128, KH, BL], F32, tag="ud")
            aeng.tensor_tensor(ud[:], sig[:, NC_:NRU, :], dd[:], OP.mult)
            # bf16 h for the next step's matmuls FIRST (it gates the PE),
            # then the fp32 hist/output copy off the critical path
            hb = hpool.tile([128, KH, BL], BF16, tag="hb")
            aeng.tensor_tensor(hb[:], h_prev, ud[:], OP.add)
            aeng.tensor_tensor(h_out, h_prev, ud[:], OP.add)
            return hb

        def flush_block(hist, tb_expr):
            for k in range(KH):
                tp = tpool.tile([128, 128], F32, tag="ftp")
                nc.tensor.transpose(tp[:], hist[:, k, :, :], identity[:])
                if k % 2 == 0:
                    nc.scalar.copy(obuf[:, k, :], tp[:])
                else:
                    nc.vector.tensor_copy(obuf[:, k, :], tp[:])
            nc.sync.dma_start(
                out_d[:, ds(tb_expr, BLK), :].transpose([1, 0, 2]),
                obuf[:].rearrange("p k n -> p (k n)"))

        def _emit_body(ib):
            hb = hpool.tile([128, KH, BL], BF16, tag="hb")
            nc.vector.tensor_copy(hb[:], histQ[:, :, BLK - 1, :])
            nc.sync.dma_start(
                xfB[:],
                zx_d[:, :, ds((ib + CHUNK) * BL, CHUNK * BL)].transpose([1, 0, 2]))
            for half in range(2):
                xf = (xfA, xfB)[half]
                for blk in range(4):
                    gblk = half * 4 + blk
                    hist = (histP, histQ)[gblk % 2]
                    prev_hist = (histP, histQ)[(gblk + 1) % 2]
                    for s in range(BLK):
                        cstep = blk * BLK + s
                        h_prev = (hist[:, :, s - 1, :] if s > 0
                                  else prev_hist[:, :, BLK - 1, :])
                        hb = emit_step(h_prev, hist[:, :, s, :], hb, xf, cstep)
                    flush_block(hist, ib + gblk * BLK)
            nc.sync.dma_start(
                xfA[:],
                zx_d[:, :, ds((ib + 2 * CHUNK) * BL, CHUNK * BL)].transpose([1, 0, 2]))

        if sim_steps is not None:
            for ib2 in range(0, sim_steps, STEPS_PER_BODY):
                _emit_body(ib2)
        else:
            with tc.For_i(0, T, STEPS_PER_BODY,
                          hint_engines=(mybir.EngineType.PE,
                                        mybir.EngineType.DVE,
                                        mybir.EngineType.Activation,
                                        mybir.EngineType.Pool)) as ib:
                _emit_body(ib)

    nc.compile()
    return nc


_CACHE = {}
LAST_RESULT = None


def _get_program(general_ln: bool):
    if general_ln not in _CACHE:
        _CACHE[general_ln] = _build_program(general_ln)
    return _CACHE[general_ln]


def build_in_maps(inputs):
    return _prep(**inputs)[0]


def _prep(x, W_xr, W_xu, W_xc, W_hr, W_hu, W_hc, h0,
          ln_xru_scale, ln_xru_bias, ln_hru_scale, ln_hru_bias,
          ln_xc_scale, ln_xc_bias, ln_hc_scale, ln_hc_bias):
    x = np.ascontiguousarray(np.asarray(x, np.float32))
    wx = np.concatenate([W_xr, W_xu, W_xc], axis=1).astype(np.float32)
    wh = np.concatenate([W_hr, W_hu, W_hc], axis=1).astype(np.float32)
    whb = np.ascontiguousarray(wh.astype(ml_dtypes.bfloat16))
    whf = whb.astype(np.float32)
    whsb = np.stack([whf[:, :2 * H].sum(1) / (2 * H),
                     whf[:, 2 * H:].sum(1) / H], axis=1)
    whsb = np.ascontiguousarray(whsb.astype(ml_dtypes.bfloat16))

    gx_full = np.concatenate([ln_xru_scale, ln_xc_scale]).astype(np.float32)
    bx_full = np.concatenate([ln_xru_bias, ln_xc_bias]).astype(np.float32)
    gh_full = np.concatenate([ln_hru_scale, ln_hc_scale]).astype(np.float32)
    bh_full = np.concatenate([ln_hru_bias, ln_hc_bias]).astype(np.float32)
    general_ln = not (np.all(gx_full == 1) and np.all(bx_full == 0)
                      and np.all(gh_full == 1) and np.all(bh_full == 0))

    h0 = np.asarray(h0, np.float32)
    h0t = np.repeat(h0.reshape(KH, 128).T[:, :, None], BL, axis=2)
    h0t = np.ascontiguousarray(h0t.reshape(128, KH * BL), np.float32)

    ident = np.eye(128, dtype=np.float32)

    shared = {
        "wx": np.ascontiguousarray(wx), "whb": whb, "whsb": whsb,
        "h0t": h0t, "ident": ident,
    }
    if general_ln:
        shared["gx"] = np.broadcast_to(gx_full, (128, H3)).copy()
        shared["bx"] = np.broadcast_to(bx_full, (128, H3)).copy()
        shared["gh"] = np.ascontiguousarray(gh_full.reshape(NT, 128).T)
        shared["bh"] = np.ascontiguousarray(bh_full.reshape(NT, 128).T)

    in_maps = []
    for c in range(NCORES):
        xl = x[c * BL:(c + 1) * BL]                      # [BL, T, D]
        xT = np.ascontiguousarray(
            xl.transpose(2, 1, 0).reshape(D, ROWS), np.float32)
        in_maps.append({"xT": xT, **shared})

    return in_maps, general_ln


def kernel(**inputs):
    in_maps, general_ln = _prep(**inputs)
    nc = _get_program(general_ln)
    res = run_bass_kernel_spmd(nc, in_maps, list(range(NCORES)))
    global LAST_RESULT
    LAST_RESULT = res
    outs = [res.results[c]["out"] for c in range(NCORES)]
    return np.concatenate(outs, axis=0).astype(np.float32)


if __name__ == "__main__":
    rng = np.random.default_rng(0)
    ins = {
        "x": rng.standard_normal((B, T, D), dtype=np.float32),
        "W_xr": rng.standard_normal((D, H), dtype=np.float32) / np.sqrt(D),
        "W_xu": rng.standard_normal((D, H), dtype=np.float32) / np.sqrt(D),
        "W_xc": rng.standard_normal((D, H), dtype=np.float32) / np.sqrt(D),
        "W_hr": rng.standard_normal((H, H), dtype=np.float32) / np.sqrt(H),
        "W_hu": rng.standard_normal((H, H), dtype=np.float32) / np.sqrt(H),
        "W_hc": rng.standard_normal((H, H), dtype=np.float32) / np.sqrt(H),
        "h0": np.zeros(H, np.float32),
        "ln_xru_scale": np.ones(2 * H, np.float32),
        "ln_xru_bias": np.zeros(2 * H, np.float32),
        "ln_hru_scale": np.ones(2 * H, np.float32),
        "ln_hru_bias": np.zeros(2 * H, np.float32),
        "ln_xc_scale": np.ones(H, np.float32),
        "ln_xc_bias": np.zeros(H, np.float32),
        "ln_hc_scale": np.ones(H, np.float32),
        "ln_hc_bias": np.zeros(H, np.float32),
    }
    out = kernel(**ins)
    print(out.shape, out.dtype, np.abs(out).mean())


# revision 5
# speedup vs baseline: 1.8023x; 1.0074x over previous
"""LayerNorm-GRU Trainium2 kernel, v2.

B=64, T=512, D=256, H=512. Data-parallel over batch: 8 rows/core x 8 cores.

Phase 1: x-side projections in fp32r (full fp32 data, 1 cyc/row on PE),
         LayerNorm (bn_stats), PE-transpose to DRAM zx [12, 128, T*8]
         feature-major.
Phase 2: recurrence, feature-major, 8 batch rows per core. Per step:
         - PE: 48 bf16 matmuls (stationary weight tiles [128,128] bf16 ->
           fast-weight-load), ru tiles first then c tiles, z in PSUM.
         - bridge: z copy PSUM->SBUF on DVE, z^2 via ACT Square (parallel);
           per LN group so the ru chain starts before the c matmuls finish.
         - stats: DVE strided reduce over feature tiles, then (stats_engine
           'pe') a ones-column matmul for the cross-partition sums, a tiny
           DVE chain computing mean/var and 1/sqrt(var+eps) via the quake
           bitwise seed + one Newton step (no ACT Sqrt -> the single
           sigmoid/tanh/square/copy table set stays resident, zero table
           reloads), and a 1x128 ones matmul broadcasting the per-batch
           stats to all partitions.
         - apply/gates: DVE normalize + gate arithmetic; ACT sigmoid/tanh.
         Output h_t accumulates in SBUF, PE-transposed to row-major and
         DMA'd out every 16 steps.
"""

import os
import sys

for _p in ("/opt/trn_rl_repo", "/root/.axon_site/_ro/trn_rl_repo"):
    if os.path.isdir(_p) and _p not in sys.path:
        sys.path.insert(0, _p)

import numpy as np
import ml_dtypes
from contextlib import ExitStack

import concourse.bass as bass
import concourse.mybir as mybir
import concourse.tile as tile
from concourse import bacc
from concourse.bass import ds
from concourse.bass_utils import run_bass_kernel_spmd

F32 = mybir.dt.float32
F32R = mybir.dt.float32r
BF16 = mybir.dt.bfloat16
I32 = mybir.dt.int32
AX = mybir.AxisListType
OP = mybir.AluOpType
AF = mybir.ActivationFunctionType
RED = bass.bass_isa.ReduceOp

B, T, D, H = 64, 512, 256, 512
NCORES = 8
BL = B // NCORES          # 8 batch rows per core
H3 = 3 * H                # 1536
NT = H3 // 128            # 12 feature tiles
NRU = (2 * H) // 128      # 8 tiles in the r|u LN group
NC_ = H // 128            # 4 tiles in the c LN group
KH = H // 128             # 4 contraction chunks for the h-matmul
ROWS = T * BL             # 4096 rows (t-major: row = t*BL + b)
EPS = 1e-5

STEPS_PER_BODY = 128
BLK = 16                  # hist flush granularity
CHUNK = 64                # steps per xfeed chunk

MAGIC = 0x5F3759DF        # quake rsqrt seed constant
NEWTON_ITERS = 1

# engine for the scalar stats chain and for the gate arithmetic
CHAIN_ENGINE = "vector"   # 'pool' | 'vector'
APPLY_ENGINE = "vector"   # 'pool' | 'vector'
# cross-partition reduction/broadcast: gpsimd all-reduce vs PE matmuls
STATS_ENGINE = "pe"       # 'pool' | 'pe'


def _build_program(general_ln: bool, sim_steps=None,
                   chain_engine=CHAIN_ENGINE, apply_engine=APPLY_ENGINE,
                   newton_iters=NEWTON_ITERS, stats_engine=STATS_ENGINE):
    nc = bacc.Bacc("TRN2", target_bir_lowering=False, debug=False)

    xT_d = nc.dram_tensor("xT", [D, ROWS], F32R, kind="ExternalInput")
    wx_d = nc.dram_tensor("wx", [D, H3], F32R, kind="ExternalInput")
    whb_d = nc.dram_tensor("whb", [H, H3], BF16, kind="ExternalInput")
    whsb_d = nc.dram_tensor("whsb", [H, 2], BF16, kind="ExternalInput")
    h0t_d = nc.dram_tensor("h0t", [128, KH * BL], F32, kind="ExternalInput")
    ident_d = nc.dram_tensor("ident", [128, 128], F32, kind="ExternalInput")
    if general_ln:
        gx_d = nc.dram_tensor("gx", [128, H3], F32, kind="ExternalInput")
        bx_d = nc.dram_tensor("bx", [128, H3], F32, kind="ExternalInput")
        gh_d = nc.dram_tensor("gh", [128, NT], F32, kind="ExternalInput")
        bh_d = nc.dram_tensor("bh", [128, NT], F32, kind="ExternalInput")
    out_d = nc.dram_tensor("out", [BL, T, H], F32, kind="ExternalOutput")
    zx_d = nc.dram_tensor("zx", [NT, 128, ROWS + CHUNK * BL], F32,
                          kind="Internal")

    with tile.TileContext(nc) as tc, ExitStack() as ctx:
        const_pool = ctx.enter_context(tc.tile_pool(name="consts", bufs=1))
        whs = const_pool.tile([128, KH, H3], BF16)
        identity = const_pool.tile([128, 128], F32)
        epsc = const_pool.tile([128, 1], F32)
        h0t = const_pool.tile([128, KH, BL], F32)
        onescol = const_pool.tile([128, 1], F32)
        ones1 = const_pool.tile([1, 128], F32)
        onescl = const_pool.tile([128, 2], F32)   # 1/N per LN group
        whsums = const_pool.tile([128, KH, 2], BF16)
        nc.vector.memset(onescol[:], 1.0)
        nc.vector.memset(ones1[:], 1.0)
        nc.vector.memset(onescl[:, 0:1], 1.0 / (2 * H))
        nc.vector.memset(onescl[:, 1:2], 1.0 / H)
        nc.sync.dma_start(whsums[:],
                          whsb_d[:].rearrange("(k p) n -> p k n", p=128))
        if general_ln:
            gx = const_pool.tile([128, H3], F32)
            bx = const_pool.tile([128, H3], F32)
            gh = const_pool.tile([128, NT], F32)
            bh = const_pool.tile([128, NT], F32)

        nc.sync.dma_start(whs[:], whb_d[:].rearrange("(k p) n -> p k n", p=128))
        nc.sync.dma_start(identity[:], ident_d[:])
        nc.sync.dma_start(h0t[:], h0t_d[:].rearrange("p (k b) -> p k b", k=KH))
        nc.vector.memset(epsc[:], EPS)
        if general_ln:
            nc.sync.dma_start(gx[:], gx_d[:])
            nc.sync.dma_start(bx[:], bx_d[:])
            nc.sync.dma_start(gh[:], gh_d[:])
            nc.sync.dma_start(bh[:], bh_d[:])

        # ================= Phase 1: x-side projections =================
        with tc.tile_pool(name="p1sbuf", bufs=1) as p1pool, \
             tc.tile_pool(name="p1work", bufs=3) as p1work, \
             tc.tile_pool(name="p1z", bufs=2, space="PSUM") as p1z, \
             tc.tile_pool(name="p1t", bufs=2, space="PSUM") as p1t:
            xts = p1pool.tile([128, 2, ROWS], F32R)
            wxs = p1pool.tile([128, 2, H3], F32R)
            nc.sync.dma_start(xts[:], xT_d[:].rearrange("(k p) n -> p k n", p=128))
            nc.sync.dma_start(wxs[:], wx_d[:].rearrange("(k p) n -> p k n", p=128))

            for r in range(ROWS // 128):
                zp = p1z.tile([128, H3], F32, tag="zp")
                for k in range(2):
                    for nb in range(3):
                        nc.tensor.matmul(
                            zp[:, nb * 512:(nb + 1) * 512],
                            xts[:, k, r * 128:(r + 1) * 128],
                            wxs[:, k, nb * 512:(nb + 1) * 512],
                            start=(k == 0), stop=(k == 1),
                        )
                sixes = p1work.tile([128, 3, 6], F32, tag="sixes")
                aggr = p1work.tile([128, 2, 2], F32, tag="aggr")
                nc.vector.bn_stats(sixes[:, 0, :], zp[:, 0:512])
                nc.vector.bn_stats(sixes[:, 1, :], zp[:, 512:1024])
                nc.vector.bn_stats(sixes[:, 2, :], zp[:, 1024:1536])
                nc.vector.bn_aggr(aggr[:, 0, :], sixes[:, 0:2, :])
                nc.vector.bn_aggr(aggr[:, 1, :], sixes[:, 2, :])
                sd = p1work.tile([128, 2], F32, tag="sd")
                inv = p1work.tile([128, 2], F32, tag="inv")
                nc.scalar.activation(sd[:], aggr[:, :, 1], AF.Sqrt, bias=epsc[:])
                nc.vector.reciprocal(inv[:], sd[:])
                zln = p1work.tile([128, H3], F32, tag="zln")
                nc.vector.tensor_scalar(
                    zln[:, 0:1024], zp[:, 0:1024],
                    aggr[:, 0, 0:1], inv[:, 0:1], OP.subtract, OP.mult)
                nc.vector.tensor_scalar(
                    zln[:, 1024:1536], zp[:, 1024:1536],
                    aggr[:, 1, 0:1], inv[:, 1:2], OP.subtract, OP.mult)
                if general_ln:
                    nc.vector.tensor_mul(zln[:], zln[:], gx[:])
                    nc.vector.tensor_add(zln[:], zln[:], bx[:])
                if r % 2 == 0:
                    ztp = p1work.tile([128, NT, 2, 128], F32, tag="ztp")
                for m in range(NT):
                    tp = p1t.tile([128, 128], F32, tag="tp")
                    nc.tensor.transpose(tp[:], zln[:, m * 128:(m + 1) * 128],
                                        identity[:])
                    # DVE is the phase-1 bottleneck (bn_stats + LN apply);
                    # route most PSUM->SBUF staging copies to ACT instead.
                    if m % 4 == 3:
                        nc.vector.tensor_copy(ztp[:, m, r % 2, :], tp[:])
                    else:
                        nc.scalar.copy(ztp[:, m, r % 2, :], tp[:])
                if r % 2 == 1:
                    nc.sync.dma_start(
                        zx_d[:, :, (r - 1) * 128:(r + 1) * 128]
                        .transpose([1, 0, 2]),
                        ztp[:].rearrange("p t two n -> p t (two n)"))

        # ================= Phase 2: recurrence =================
        xfA = const_pool.tile([128, NT, CHUNK * BL], F32)
        xfB = const_pool.tile([128, NT, CHUNK * BL], F32)
        histP = const_pool.tile([128, KH, BLK, BL], F32)
        histQ = const_pool.tile([128, KH, BLK, BL], F32)
        obuf = const_pool.tile([128, KH, 128], F32)

        nc.vector.tensor_copy(histQ[:, :, BLK - 1, :], h0t[:])
        nc.sync.dma_start(
            xfA[:], zx_d[:, :, 0:CHUNK * BL].transpose([1, 0, 2]))

        zpool = ctx.enter_context(tc.tile_pool(name="zp2", bufs=2, space="PSUM"))
        spool = ctx.enter_context(tc.tile_pool(name="sp2", bufs=2, space="PSUM"))
        tpool = ctx.enter_context(tc.tile_pool(name="tp2", bufs=2, space="PSUM"))
        wpool = ctx.enter_context(tc.tile_pool(name="w2", bufs=3))
        hpool = ctx.enter_context(tc.tile_pool(name="hb2", bufs=3))

        ceng = {"pool": nc.gpsimd, "vector": nc.vector}[chain_engine]
        aeng = {"pool": nc.gpsimd, "vector": nc.vector}[apply_engine]

        def chain_ops(P, src_sums, n_feat, g, sb=None, goff=0):
            """Mean/var/quake-rsqrt on [P, BL] tiles from src_sums
            ([P, 2, BL]: z-sums | sq-sums). Returns (y_ap, mis_ap) as
            [P, BL] APs (for 'pe', written into SBUF stats tile)."""
            mm = wpool.tile([P, BL], F32, tag=f"mm{g}")
            ceng.tensor_scalar(mm[:], src_sums[:, 0, :], 1.0 / n_feat, None,
                               OP.mult)
            msq = wpool.tile([P, BL], F32, tag=f"msq{g}")
            ceng.tensor_tensor(msq[:], mm[:], mm[:], OP.mult)
            ve = wpool.tile([P, BL], F32, tag=f"ve{g}")
            ceng.tensor_scalar(ve[:], src_sums[:, 1, :], 1.0 / n_feat, EPS,
                               OP.mult, OP.add)
            v = wpool.tile([P, BL], F32, tag=f"v{g}")
            ceng.tensor_tensor(v[:], ve[:], msq[:], OP.subtract)
            # quake seed: one fused DVE op computes ~(i >> 1) (bitwise ops
            # are illegal on Pool); then an int add gives MAGIC - (i >> 1).
            nt_ = wpool.tile([P, BL], I32, tag=f"nt{g}")
            nc.vector.tensor_scalar(nt_[:], v[:].bitcast(I32), 1, -1,
                                    OP.logical_shift_right, OP.bitwise_xor)
            y_t = wpool.tile([P, BL], F32, tag=f"y{g}")
            y = y_t[:]
            ceng.tensor_scalar(y.bitcast(I32), nt_[:], MAGIC + 1, None,
                               OP.add)
            for it in range(newton_iters):
                a = wpool.tile([P, BL], F32, tag=f"qa{g}_{it}")
                ceng.tensor_tensor(a[:], y, y, OP.mult)
                w_ = wpool.tile([P, BL], F32, tag=f"qw{g}_{it}")
                ceng.tensor_tensor(w_[:], v[:], a[:], OP.mult)
                f_ = wpool.tile([P, BL], F32, tag=f"qf{g}_{it}")
                ceng.tensor_scalar(f_[:], w_[:], -0.5, 1.5, OP.mult, OP.add)
                last = it == newton_iters - 1
                if last and sb is not None:
                    y2 = sb[0:1, 0:BL]
                else:
                    y2_t = wpool.tile([P, BL], F32, tag=f"qy{g}_{it}")
                    y2 = y2_t[:]
                ceng.tensor_tensor(y2, y, f_[:], OP.mult)
                y = y2
            if sb is not None:
                mis = sb[0:1, BL:2 * BL]
            else:
                mis_t = wpool.tile([P, BL], F32, tag=f"mis{g}")
                mis = mis_t[:]
            ceng.tensor_tensor(mis, mm[:], y, OP.mult)
            return y, mis

        def group_chain(g, gi, zq, n_feat, ntiles, sbp):
            """Stats for one LN group. zq: SBUF [128, 2, ntiles, BL]
            (z | z^2). Returns (y_bc, mis_bc) as [128, BL] APs replicated
            on all partitions (SBUF for 'pool', PSUM for 'pe')."""
            if stats_engine == "pool":
                ps = wpool.tile([128, 2, BL], F32, tag=f"ps{g}")
                nc.vector.tensor_reduce(
                    ps[:], zq[:].rearrange("p c t b -> p c b t"), AX.X, OP.add)
                allr = wpool.tile([128, 2, BL], F32, tag=f"allr{g}")
                nc.gpsimd.partition_all_reduce(
                    allr[:].rearrange("p c b -> p (c b)"),
                    ps[:].rearrange("p c b -> p (c b)"),
                    channels=128, reduce_op=RED.add)
                y, mis = chain_ops(128, allr, n_feat, g)
                return y, mis
            # 'pe': the group mean is already accumulating in
            # sbp[0:1, gi*BL:(gi+1)*BL] via the pre-scaled folded weight
            # columns (part of the PE matmul phase); only sum(z^2) needs the
            # reduce + ones-matmul (the ones column is pre-scaled by 1/N).
            psq = wpool.tile([128, BL], F32, tag=f"ps{g}")
            nc.vector.tensor_reduce(
                psq[:], zq[:, 1, :, :].rearrange("p t b -> p b t"),
                AX.X, OP.add)
            nc.tensor.matmul(
                sbp[0:1, (2 + gi) * BL:(3 + gi) * BL], onescl[:, gi:gi + 1],
                psq[:], start=True, stop=True)
            mcp = wpool.tile([1, BL], F32, tag=f"mcp{g}")
            nc.vector.tensor_copy(mcp[:], sbp[0:1, gi * BL:(gi + 1) * BL])
            msq = wpool.tile([1, BL], F32, tag=f"msq{g}")
            ceng.tensor_tensor(msq[:], mcp[:], mcp[:], OP.mult)
            v = wpool.tile([1, BL], F32, tag=f"v{g}")
            nc.vector.scalar_tensor_tensor(
                v[:], sbp[0:1, (2 + gi) * BL:(3 + gi) * BL], EPS, msq[:],
                OP.add, OP.subtract)
            nt_ = wpool.tile([1, BL], I32, tag=f"nt{g}")
            nc.vector.tensor_scalar(nt_[:], v[:].bitcast(I32), 1, -1,
                                    OP.logical_shift_right, OP.bitwise_xor)
            st = wpool.tile([1, 2 * BL], F32, tag=f"st{g}")
            y_t = wpool.tile([1, BL], F32, tag=f"yq{g}")
            y = y_t[:]
            ceng.tensor_scalar(y.bitcast(I32), nt_[:], MAGIC + 1, None,
                               OP.add)
            for it in range(newton_iters):
                a = wpool.tile([1, BL], F32, tag=f"qa{g}_{it}")
                ceng.tensor_tensor(a[:], y, y, OP.mult)
                f_ = wpool.tile([1, BL], F32, tag=f"qf{g}_{it}")
                nc.vector.scalar_tensor_tensor(f_[:], a[:], -0.5, v[:],
                                               OP.mult, OP.mult)
                y2 = (st[0:1, 0:BL] if it == newton_iters - 1
                      else None)
                if y2 is None:
                    y2_t = wpool.tile([1, BL], F32, tag=f"qy{g}_{it}")
                    y2 = y2_t[:]
                nc.vector.scalar_tensor_tensor(y2, f_[:], 1.5, y,
                                               OP.add, OP.mult)
                y = y2
            ceng.tensor_tensor(st[0:1, BL:2 * BL], mcp[:], y, OP.mult)
            goff = (4 + 2 * gi) * BL
            nc.tensor.matmul(
                sbp[:, goff:goff + 2 * BL], ones1[0:1, :], st[0:1, :],
                start=True, stop=True)
            return (sbp[:, goff:goff + BL],
                    sbp[:, goff + BL:goff + 2 * BL])

        def emit_step(h_prev, h_out, hb_prev, xf, cstep):
            """One GRU step. h_prev/h_out: [128, KH, BL] APs (feature-major).
            hb_prev: [128, KH, BL] bf16 tile; returns the next hb tile."""
            zru = zpool.tile([128, NRU, BL], F32, tag="zru")
            zc = zpool.tile([128, NC_, BL], F32, tag="zc")
            sbp = None
            if stats_engine == "pe":
                sbp = spool.tile([128, 8 * BL], F32, tag="sb")
            for m in range(NRU):
                for k in range(KH):
                    nc.tensor.matmul(
                        zru[:, m, :], whs[:, k, m * 128:(m + 1) * 128],
                        hb_prev[:, k, :], start=(k == 0), stop=(k == KH - 1))
            if stats_engine == "pe":
                # group means ride along as two extra matmul columns against
                # the pre-scaled folded weight sums
                for gi in range(2):
                    for k in range(KH):
                        nc.tensor.matmul(
                            sbp[0:1, gi * BL:(gi + 1) * BL],
                            whsums[:, k, gi:gi + 1], hb_prev[:, k, :],
                            start=(k == 0), stop=(k == KH - 1))
            for m in range(NC_):
                for k in range(KH):
                    nc.tensor.matmul(
                        zc[:, m, :], whs[:, k, (NRU + m) * 128:(NRU + m + 1) * 128],
                        hb_prev[:, k, :], start=(k == 0), stop=(k == KH - 1))

            # bridge PSUM -> SBUF: z copy on DVE, square on ACT (parallel)
            zqru = wpool.tile([128, 2, NRU, BL], F32, tag="zqru")
            nc.vector.tensor_copy(
                zqru[:, 0, :, :].rearrange("p t b -> p (t b)"),
                zru[:].rearrange("p t b -> p (t b)"))
            nc.scalar.activation(
                zqru[:, 1, :, :].rearrange("p t b -> p (t b)"),
                zru[:].rearrange("p t b -> p (t b)"), AF.Square)
            zqc = wpool.tile([128, 2, NC_, BL], F32, tag="zqc")
            nc.vector.tensor_copy(
                zqc[:, 0, :, :].rearrange("p t b -> p (t b)"),
                zc[:].rearrange("p t b -> p (t b)"))
            nc.scalar.activation(
                zqc[:, 1, :, :].rearrange("p t b -> p (t b)"),
                zc[:].rearrange("p t b -> p (t b)"), AF.Square)

            y_ru, mis_ru = group_chain("r", 0, zqru, 2.0 * H, NRU, sbp)

            xs = xf[:, :, cstep * BL:(cstep + 1) * BL]
            # ru apply: pre = z*is + (x - mis)  (emitted BEFORE the c-group
            # chain so the scheduler runs the c chain during sigmoid, not
            # ahead of the critical ru-apply path)
            xm = wpool.tile([128, NRU, BL], F32, tag="xm")
            aeng.tensor_tensor(
                xm[:], xs[:, 0:NRU, :],
                mis_ru.unsqueeze(1).to_broadcast([128, NRU, BL]),
                OP.subtract)
            tru = wpool.tile([128, NRU, BL], F32, tag="tru")
            aeng.tensor_tensor(
                tru[:], zqru[:, 0, :, :],
                y_ru.unsqueeze(1).to_broadcast([128, NRU, BL]), OP.mult)
            if general_ln:
                nc.vector.tensor_mul(
                    tru[:], tru[:],
                    gh[:, 0:NRU].unsqueeze(2).to_broadcast([128, NRU, BL]))
                gmis = wpool.tile([128, NRU, BL], F32, tag="gmis")
                nc.vector.tensor_tensor(
                    gmis[:],
                    mis_ru.unsqueeze(1).to_broadcast([128, NRU, BL]),
                    gh[:, 0:NRU].unsqueeze(2).to_broadcast([128, NRU, BL]),
                    OP.mult)
                nc.vector.tensor_tensor(
                    xm[:], xs[:, 0:NRU, :], gmis[:], OP.subtract)
                nc.vector.tensor_add(
                    xm[:], xm[:],
                    bh[:, 0:NRU].unsqueeze(2).to_broadcast([128, NRU, BL]))
            pre = wpool.tile([128, NRU, BL], F32, tag="pre")
            aeng.tensor_tensor(pre[:], tru[:], xm[:], OP.add)
            sig = wpool.tile([128, NRU, BL], F32, tag="sig")
            nc.scalar.activation(
                sig[:].rearrange("p a b -> p (a b)"),
                pre[:].rearrange("p a b -> p (a b)"), AF.Sigmoid)

            y_c, mis_c = group_chain("c", 1, zqc, float(H), NC_, sbp)
            # c apply
            tc_ = wpool.tile([128, NC_, BL], F32, tag="tc_")
            aeng.tensor_tensor(
                tc_[:], zqc[:, 0, :, :],
                y_c.unsqueeze(1).to_broadcast([128, NC_, BL]), OP.mult)
            oc = wpool.tile([128, NC_, BL], F32, tag="oc")
            aeng.tensor_tensor(
                oc[:], tc_[:],
                mis_c.unsqueeze(1).to_broadcast([128, NC_, BL]),
                OP.subtract)
            if general_ln:
                nc.vector.tensor_mul(
                    oc[:], oc[:],
                    gh[:, NRU:NT].unsqueeze(2).to_broadcast([128, NC_, BL]))
                nc.vector.tensor_add(
                    oc[:], oc[:],
                    bh[:, NRU:NT].unsqueeze(2).to_broadcast([128, NC_, BL]))
            rh = wpool.tile([128, NC_, BL], F32, tag="rh")
            aeng.tensor_tensor(rh[:], sig[:, 0:NC_, :], oc[:], OP.mult)
            prec = wpool.tile([128, NC_, BL], F32, tag="prec")
            aeng.tensor_tensor(prec[:], rh[:], xs[:, NRU:NT, :], OP.add)
            cc = wpool.tile([128, NC_, BL], F32, tag="cc")
            nc.scalar.activation(
                cc[:].rearrange("p a b -> p (a b)"),
                prec[:].rearrange("p a b -> p (a b)"), AF.Tanh)
            dd = wpool.tile([128, KH, BL], F32, tag="dd")
            aeng.tensor_tensor(dd[:], cc[:], h_prev, OP.subtract)
            ud = wpool.tile([128, KH, BL], F32, tag="ud")
            aeng.tensor_tensor(ud[:], sig[:, NC_:NRU, :], dd[:], OP.mult)
            # bf16 h for the next step's matmuls FIRST (it gates the PE),
            # then the fp32 hist/output copy off the critical path
            hb = hpool.tile([128, KH, BL], BF16, tag="hb")
            aeng.tensor_tensor(hb[:], h_prev, ud[:], OP.add)
            aeng.tensor_tensor(h_out, h_prev, ud[:], OP.add)
            return hb

        def flush_block(hist, tb_expr):
            for k in range(KH):
                tp = tpool.tile([128, 128], F32, tag="ftp")
                nc.tensor.transpose(tp[:], hist[:, k, :, :], identity[:])
                if k % 2 == 0:
                    nc.scalar.copy(obuf[:, k, :], tp[:])
                else:
                    nc.vector.tensor_copy(obuf[:, k, :], tp[:])
            nc.sync.dma_start(
                out_d[:, ds(tb_expr, BLK), :].transpose([1, 0, 2]),
                obuf[:].rearrange("p k n -> p (k n)"))

        def _emit_body(ib):
            hb = hpool.tile([128, KH, BL], BF16, tag="hb")
            nc.vector.tensor_copy(hb[:], histQ[:, :, BLK - 1, :])
            nc.sync.dma_start(
                xfB[:],
                zx_d[:, :, ds((ib + CHUNK) * BL, CHUNK * BL)].transpose([1, 0, 2]))
            for half in range(2):
                xf = (xfA, xfB)[half]
                for blk in range(4):
                    gblk = half * 4 + blk
                    hist = (histP, histQ)[gblk % 2]
                    prev_hist = (histP, histQ)[(gblk + 1) % 2]
                    for s in range(BLK):
                        cstep = blk * BLK + s
                        h_prev = (hist[:, :, s - 1, :] if s > 0
                                  else prev_hist[:, :, BLK - 1, :])
                        hb = emit_step(h_prev, hist[:, :, s, :], hb, xf, cstep)
                    flush_block(hist, ib + gblk * BLK)
            nc.sync.dma_start(
                xfA[:],
                zx_d[:, :, ds((ib + 2 * CHUNK) * BL, CHUNK * BL)].transpose([1, 0, 2]))

        if sim_steps is not None:
            for ib2 in range(0, sim_steps, STEPS_PER_BODY):
                _emit_body(ib2)
        else:
            with tc.For_i(0, T, STEPS_PER_BODY,
                          hint_engines=(mybir.EngineType.PE,
                                        mybir.EngineType.DVE,
                                        mybir.EngineType.Activation,
                                        mybir.EngineType.Pool)) as ib:
                _emit_body(ib)

    nc.compile()
    return nc


_CACHE = {}
LAST_RESULT = None


def _get_program(general_ln: bool):
    if general_ln not in _CACHE:
        _CACHE[general_ln] = _build_program(general_ln)
    return _CACHE[general_ln]


def build_in_maps(inputs):
    return _prep(**inputs)[0]


def _prep(x, W_xr, W_xu, W_xc, W_hr, W_hu, W_hc, h0,
          ln_xru_scale, ln_xru_bias, ln_hru_scale, ln_hru_bias,
          ln_xc_scale, ln_xc_bias, ln_hc_scale, ln_hc_bias):
    x = np.ascontiguousarray(np.asarray(x, np.float32))
    wx = np.concatenate([W_xr, W_xu, W_xc], axis=1).astype(np.float32)
    wh = np.concatenate([W_hr, W_hu, W_hc], axis=1).astype(np.float32)
    whb = np.ascontiguousarray(wh.astype(ml_dtypes.bfloat16))
    whf = whb.astype(np.float32)
    whsb = np.stack([whf[:, :2 * H].sum(1) / (2 * H),
                     whf[:, 2 * H:].sum(1) / H], axis=1)
    whsb = np.ascontiguousarray(whsb.astype(ml_dtypes.bfloat16))

    gx_full = np.concatenate([ln_xru_scale, ln_xc_scale]).astype(np.float32)
    bx_full = np.concatenate([ln_xru_bias, ln_xc_bias]).astype(np.float32)
    gh_full = np.concatenate([ln_hru_scale, ln_hc_scale]).astype(np.float32)
    bh_full = np.concatenate([ln_hru_bias, ln_hc_bias]).astype(np.float32)
    general_ln = not (np.all(gx_full == 1) and np.all(bx_full == 0)
                      and np.all(gh_full == 1) and np.all(bh_full == 0))

    h0 = np.asarray(h0, np.float32)
    h0t = np.repeat(h0.reshape(KH, 128).T[:, :, None], BL, axis=2)
    h0t = np.ascontiguousarray(h0t.reshape(128, KH * BL), np.float32)

    ident = np.eye(128, dtype=np.float32)

    shared = {
        "wx": np.ascontiguousarray(wx), "whb": whb, "whsb": whsb,
        "h0t": h0t, "ident": ident,
    }
    if general_ln:
        shared["gx"] = np.broadcast_to(gx_full, (128, H3)).copy()
        shared["bx"] = np.broadcast_to(bx_full, (128, H3)).copy()
        shared["gh"] = np.ascontiguousarray(gh_full.reshape(NT, 128).T)
        shared["bh"] = np.ascontiguousarray(bh_full.reshape(NT, 128).T)

    in_maps = []
    for c in range(NCORES):
        xl = x[c * BL:(c + 1) * BL]                      # [BL, T, D]
        xT = np.ascontiguousarray(
            xl.transpose(2, 1, 0).reshape(D, ROWS), np.float32)
        in_maps.append({"xT": xT, **shared})

    return in_maps, general_ln


def kernel(**inputs):
    in_maps, general_ln = _prep(**inputs)
    nc = _get_program(general_ln)
    res = run_bass_kernel_spmd(nc, in_maps, list(range(NCORES)))
    global LAST_RESULT
    LAST_RESULT = res
    outs = [res.results[c]["out"] for c in range(NCORES)]
    return np.concatenate(outs, axis=0).astype(np.float32)


if __name__ == "__main__":
    rng = np.random.default_rng(0)
    ins = {
        "x": rng.standard_normal((B, T, D), dtype=np.float32),
        "W_xr": rng.standard_normal((D, H), dtype=np.float32) / np.sqrt(D),
        "W_xu": rng.standard_normal((D, H), dtype=np.float32) / np.sqrt(D),
        "W_xc": rng.standard_normal((D, H), dtype=np.float32) / np.sqrt(D),
        "W_hr": rng.standard_normal((H, H), dtype=np.float32) / np.sqrt(H),
        "W_hu": rng.standard_normal((H, H), dtype=np.float32) / np.sqrt(H),
        "W_hc": rng.standard_normal((H, H), dtype=np.float32) / np.sqrt(H),
        "h0": np.zeros(H, np.float32),
        "ln_xru_scale": np.ones(2 * H, np.float32),
        "ln_xru_bias": np.zeros(2 * H, np.float32),
        "ln_hru_scale": np.ones(2 * H, np.float32),
        "ln_hru_bias": np.zeros(2 * H, np.float32),
        "ln_xc_scale": np.ones(H, np.float32),
        "ln_xc_bias": np.zeros(H, np.float32),
        "ln_hc_scale": np.ones(H, np.float32),
        "ln_hc_bias": np.zeros(H, np.float32),
    }
    out = kernel(**ins)
    print(out.shape, out.dtype, np.abs(out).mean())
